# revision 32
# baseline (speedup 1.0000x reference)
"""Trainium2 Bass kernel for prefix-KV multi-head attention (v2).

Reference computation (per batch):
    qkv = x @ w_qkv -> q,k,v heads; k/v get a 16-token prefix (pk, pv)
    attn = softmax(q @ k^T * D^-0.5); out = (attn @ v) @ w_proj + b_proj

Sharding: data-parallel over B across 8 NeuronCores (2 batches per core).

Design (vs the v1 baseline, 700us -> 608us):
  - weights loaded to SBUF once per core (bf16), reused by both batches
  - q^T kept in SBUF (no DRAM spill)
  - v computed in NATURAL [token, feature] layout via x^T-stationary GEMM
    (moving = w_v columns), eliminating all per-head v transposes
  - x^T built with bf16 PE transposes (2x faster than fp32)
  - attention runs per HEAD (not head-pair): PSUM = scores 2x2 banks
    (double buffered) + av accumulator 2 banks + gemm scratch 2x1 banks
    = 8 banks exactly
  - q/k/v GEMM chunks for pair p+1 and proj passes of the previous batch
    are software-pipelined into the attention mt-loop slots, so the PE
    stays busy while ACT computes exp()
  - softmax 1/denominator via exp(-ln(d)) on ACT (this walrus lacks the
    custom-DVE approx ops; iterative DVE reciprocal costs 6.5us); a DVE
    copy of the numerator releases the av psum accumulator early
  - ones-columns packed next to v in v_ext give the softmax denominator
    for free inside the attention@v matmul (rows 64:128 of the psum)

Explored and rejected (all measured on HW): fp8 (2e-2 tolerance
exceeded: random-sign GEMM error stays ~5.7% relative regardless of N);
PE tile-packing of the K=64 score matmuls (verified ~1.9x overlap on
alternating-row-half pairs via microbenchmark, but a full-array matmul
issued behind a packed pair corrupts the array unless sync-guarded, and
guarded variants measured 641-759us vs 608us -- see kernel_v5.py /
kernel_v4_packed.py); normalize multiply on the Pool engine (Pool
tensor ops ~3x slower than modeled: 742us); deferring the normalize mul
by one head (624us); 1024-col moving matmuls (hardware ISA caps moving
at 512).

This file is self-contained: it monkeypatches two workarounds for the
walrus build in this container (1-sync-wait-per-instruction cap).
"""

import json
import os
import sys
from collections import deque

for _p in ("/opt/trn_rl_repo", os.path.expanduser("~/.axon_site/_ro/trn_rl_repo")):
    if os.path.isdir(_p) and _p not in sys.path:
        sys.path.insert(0, _p)

import numpy as np

import concourse.bass as bass
import concourse.tile as tile
from concourse import mybir
from concourse.bass_utils import run_bass_kernel_spmd
from concourse.vector_clock import ScopedClock
from concourse.masks import make_identity

F32 = mybir.dt.float32
BF16 = mybir.dt.bfloat16
AF = mybir.ActivationFunctionType

# ---------------------------------------------------------------------------
# Workaround: this container's walrus supports at most ONE sync wait per
# instruction.  (a) split the TileContext-exit drain's waits onto single-wait
# NOPs; (b) at BIR-JSON serialization time, hoist extra waits from any
# instruction onto same-engine NOPs placed immediately before it.
# ---------------------------------------------------------------------------

def _patched_drain_and_barrier(self, tick_clock, wait_clock):
    drain_inst = self.nc.sync.drain()
    wait_clock.add_sem_waits(
        drain_inst.ins, ScopedClock({None: tick_clock.global_clock})
    )
    si = drain_inst.ins.sync_info
    waits = list(si.on_wait) if si is not None and si.on_wait else []
    if len(waits) > 1:
        si.on_wait = waits[:1]
        for w in waits[1:]:
            nop = self.nc.sync.nop(hint="drain_wait_split", nofuse=True)
            nsi = nop.ins.sync_info
            if nsi is None:
                nop.ins.sync_info = mybir.SyncInfo(on_wait=[w], on_update=[])
            else:
                nsi.on_wait = list(nsi.on_wait or []) + [w]
    self.nc.all_engine_barrier()
    assert self.sems is not None
    popped = self.nc._tile_sem_poison_stack.pop()
    assert popped is self._sem_poison
    self.nc.clear_and_free_semaphores(list(self.sems.allocated().values()))
    self.nc.all_engine_barrier()


tile.TileContext._drain_and_barrier = _patched_drain_and_barrier


def _split_multi_waits(bir):
    for fn in bir["functions"]:
        for bb in fn["blocks"]:
            new_insts = []
            for inst in bb["instructions"]:
                si = inst.get("sync_info")
                ow = (si or {}).get("on_wait") or []
                if len(ow) > 1:
                    for i, w in enumerate(ow[:-1]):
                        new_insts.append({
                            "debug": inst.get("debug", 0),
                            "engine": inst["engine"],
                            "ins": [], "outs": [],
                            "name": f"{inst['name']}.wsplit{i}",
                            "opcode": "NoOp",
                            "sync_info": {"on_wait": [w], "on_update": []},
                        })
                    si["on_wait"] = [ow[-1]]
                new_insts.append(inst)
            bb["instructions"] = new_insts
    return bir


_orig_to_json_bytes = bass.Bass.to_json_bytes


def _patched_to_json_bytes(self):
    d = json.loads(_orig_to_json_bytes(self))
    _split_multi_waits(d)
    return json.dumps(d).encode()


bass.Bass.to_json_bytes = _patched_to_json_bytes

# ---------------------------------------------------------------------------
# Problem constants (hardcoded per the task contract)
# ---------------------------------------------------------------------------

B, N, C, H, P = 16, 1024, 1024, 16, 16
D = C // H                      # 64
SCALE = float(D) ** -0.5        # 0.125
N_CORES = 8
B_PC = B // N_CORES             # 2 batches per core
NT = N // 128                   # 8 token tiles
CT = C // 128                   # 8 feature tiles
MT = NT + 1                     # 9 m-tiles: tile 0 = prefix (16 valid rows)
HPAIRS = H // 2                 # 8 head pairs
FOLD_NEXT = True


def build_nc(repeat: int = 1) -> bass.Bass:
    nc = bass.Bass()

    x_d = nc.declare_dram_parameter("x", [B_PC, N, C], F32, isOutput=False)
    pk_d = nc.declare_dram_parameter("pk", [B_PC, P, C], F32, isOutput=False)
    pv_d = nc.declare_dram_parameter("pv", [B_PC, P, C], F32, isOutput=False)
    wqkv_d = nc.declare_dram_parameter("w_qkv", [C, 3 * C], F32, isOutput=False)
    wproj_d = nc.declare_dram_parameter("w_proj", [C, C], F32, isOutput=False)
    bias_d = nc.declare_dram_parameter("b_proj", [C], F32, isOutput=False)
    # output is stored TRANSPOSED per batch: [C, N]; host transposes back
    outT_d = nc.declare_dram_parameter("outT", [B_PC, C, N], F32, isOutput=True)

    with tile.TileContext(nc) as tc:
        with tc.tile_pool(name="cons", bufs=1) as cons, \
             tc.tile_pool(name="eP", bufs=3) as e_pool, \
             tc.tile_pool(name="ePre", bufs=1) as epre_pool, \
             tc.tile_pool(name="stg", bufs=1) as stg, \
             tc.tile_pool(name="rbp", bufs=1) as rb_pool, \
             tc.tile_pool(name="xload", bufs=2) as xload, \
             tc.tile_pool(name="xbf", bufs=2) as xbfp, \
             tc.tile_pool(name="osb", bufs=2) as osb, \
             tc.tile_pool(name="psS", bufs=2, space="PSUM") as psS, \
             tc.tile_pool(name="psAV", bufs=1, space="PSUM") as psAV, \
             tc.tile_pool(name="psG", bufs=2, space="PSUM") as psG:

            # ---------------- one-time setup ----------------
            ident_bf = cons.tile([128, 128], BF16, tag="idb")
            make_identity(nc, ident_bf[:])
            # PE warm-up burst: ~3.5us of throwaway matmuls releases the
            # HAM clock-gate (K=4/8 -> 8/8) before the real work arrives,
            # so the preamble transposes/GEMMs run at 2.4 GHz not 1.2.
            warm_ps = psG.tile([128, 128], F32, tag="g", name="warmup")
            for _w in range(32):
                nc.tensor.matmul(
                    warm_ps[:], ident_bf[:], ident_bf[:],
                    start=(_w == 0), stop=(_w == 31),
                )
            # bias in per-partition layout: bias_col[p, cf] = b_proj[cf*128+p]
            bias_col = cons.tile([128, CT], F32, tag="bias")
            nc.sync.dma_start(
                out=bias_col[:],
                in_=bias_d[:].rearrange("(a b) -> b a", b=128),
            )
            # prefix-k staging (bf16 via casting gpsimd DMA)
            pkl = cons.tile([P, C], BF16, tag="pkl")

            # persistent activations (reused across batches; Tile tracks
            # read/write hazards on AP ranges).  qT/kT hold THREE head
            # pairs (slot p%3): pair p+2 is produced by pipelined fillers
            # while pair p's attention reads its slot; the extra slot lets
            # the packed-prefix exp (4 heads = 2 pairs per ACTIVATE) see
            # both of its pairs' q at group start.
            xT = cons.tile([128, CT, N], BF16, tag="xT")
            kT = cons.tile([128, 4, N], BF16, tag="kT")
            qT = cons.tile([128, 4, N], BF16, tag="qT")
            # prefix keys, all pairs: cols 0:16 = pk^T, 16:32 zero so the
            # packed 32-row score stripes come out 0 on rows 16:32 ->
            # exp = 1, harmless because the matching v_ext rows are zero
            kPre = cons.tile([128, HPAIRS, 32], BF16, tag="kPre")
            nc.vector.memset(kPre[:, :, P:32], 0.0)
            oT = cons.tile([128, CT, N], BF16, tag="oT")
            # first-half (head pairs 0-3) projection partials, bf16; the A
            # pass runs as lazy filler inside the SAME batch's pairs 4+,
            # the B pass (pairs 4-7 + combine + store) carries to the next
            # batch's preamble
            o_half = cons.tile([128, CT, N], BF16, tag="oh")
            # v_ext[m, mt, h, 0:64] = v values; [.., 64:128] = ones columns
            # (denominator trick). m-tile 0 = prefix, PACKED: head h's 16
            # pv rows live at partitions 32*(h%4)..+16 (matching its stripe
            # in the packed prefix-score psum); all other rows stay ZERO so
            # the other heads' e values in the shared e_pre tile contribute
            # nothing to this head's av or denominator.
            v_ext = cons.tile([128, MT, H, 128], BF16, tag="vx")
            nc.vector.memset(v_ext[:, :, :, 64:128], 1.0)
            nc.vector.memset(v_ext[:, 0, :, :], 0.0)
            for a in range(4):
                nc.vector.memset(
                    v_ext[32 * a:32 * a + P, 0, a::4, 64:128], 1.0
                )

            # weights, bf16, resident for the whole kernel, on the gpsimd
            # sw-DGE queue (the only one that casts).  512-col chunks keep
            # the write packets at 1KB (128-col chunks made 256B packets and
            # left the queue packet-rate-bound for ~60us).  x rides the
            # separate sync HW queue concurrently.
            wq_sb = cons.tile([128, CT, C], BF16, tag="wq")
            wk_sb = cons.tile([128, CT, C], BF16, tag="wk")
            wv_sb = cons.tile([128, CT, C], BF16, tag="wv")
            wp_sb = cons.tile([128, CT, C], BF16, tag="wp")

            def _wload(dst, base, lo, hi):
                nc.gpsimd.dma_start(
                    out=dst[:, :, lo:hi],
                    in_=wqkv_d[:, base + lo:base + hi].rearrange(
                        "(ct p) f -> p ct f", p=128),
                )

            def _pv_load(b):
                pvr = pv_d[b].rearrange("t (h d) -> t h d", d=64)
                for a in range(4):
                    nc.gpsimd.dma_start(
                        out=v_ext[32 * a:32 * a + P, 0, a::4, 0:64],
                        in_=pvr[:, a::4, :],
                    )

            nc.gpsimd.dma_start(out=pkl[:], in_=pk_d[0])
            _wload(wk_sb, C, 0, 128)                  # k pair 0
            _wload(wq_sb, 0, 0, 128)                  # q pair 0
            _wload(wq_sb, 0, 128, 256)                # q pair 1
            _wload(wk_sb, C, 128, 256)                # k pair 1
            _wload(wv_sb, 2 * C, 0, 512)              # v block 0
            _pv_load(0)                               # prefix v, batch 0
            _wload(wv_sb, 2 * C, 512, 1024)           # v block 1
            _wload(wk_sb, C, 256, 640)
            _wload(wq_sb, 0, 256, 640)
            _wload(wk_sb, C, 640, 1024)
            _wload(wq_sb, 0, 640, 1024)
            nc.gpsimd.dma_start(
                out=wp_sb[:],
                in_=wproj_d[:].rearrange("(ct p) f -> p ct f", p=128),
            )

            # ---------------- per-batch work units ----------------

            def qk_units(b, p):
                """4 closures: q and k GEMMs for head pair p, split in two
                512-column halves each. Each accumulates 8 c-tiles into a
                [128,512] psum and copies (cast bf16) into qT/kT."""
                us = []
                for which in ("k", "q"):
                    for jh in range(2):
                        def u(which=which, p=p, jh=jh, b=b):
                            w_sb = wk_sb if which == "k" else wq_sb
                            ps = psG.tile([128, 512], F32, tag="g",
                                          name=f"g{which}_{b}_{p}_{jh}")
                            for ct in range(CT):
                                nc.tensor.matmul(
                                    ps[:],
                                    w_sb[:, ct, p * 128:(p + 1) * 128],
                                    xT[:, ct, jh * 512:(jh + 1) * 512],
                                    start=(ct == 0), stop=(ct == CT - 1),
                                )
                            if which == "k":
                                nc.vector.tensor_copy(
                                    kT[:, p % 4, jh * 512:(jh + 1) * 512],
                                    ps[:],
                                )
                            else:
                                nc.vector.tensor_copy(
                                    qT[:, p % 4, jh * 512:(jh + 1) * 512],
                                    ps[:],
                                )
                        us.append(u)
                return us

            def v_units(b, bk):
                """8 closures: v GEMM for pair block bk (4 pairs = 512 v
                columns), one per token tile. x^T tile is stationary, w_v
                columns are moving -> v lands in NATURAL [token, feature]
                layout, no transpose needed."""
                us = []
                for nt in range(NT):
                    def u(nt=nt, bk=bk, b=b):
                        ps = psG.tile([128, 512], F32, tag="g",
                                      name=f"gv_{b}_{bk}_{nt}")
                        for ct in range(CT):
                            nc.tensor.matmul(
                                ps[:],
                                xT[:, ct, nt * 128:(nt + 1) * 128],
                                wv_sb[:, ct, bk * 512:(bk + 1) * 512],
                                start=(ct == 0), stop=(ct == CT - 1),
                            )
                        nc.vector.tensor_copy(
                            v_ext[:, nt + 1, 8 * bk:8 * (bk + 1), 0:64],
                            ps[:].rearrange("p (h d) -> p h d", d=64),
                        )
                    us.append(u)
                return us

            def projA_units(b):
                """8 closures: projection over oT head-pairs 0-3 (+bias)
                into bf16 o_half.  Ready once pair 3 is normalized, so they
                fill THIS batch's pair 4+ attention slots."""
                us = []
                for cf in range(CT):
                    def u(cf=cf, b=b):
                        ps = psS.tile([128, N], F32, tag="s",
                                      name=f"pa_{b}_{cf}")
                        for ct in range(4):
                            for j in (0, 512):
                                nc.tensor.matmul(
                                    ps[:, j:j + 512],
                                    wp_sb[:, ct, cf * 128:(cf + 1) * 128],
                                    oT[:, ct, j:j + 512],
                                    start=(ct == 0), stop=(ct == 3),
                                )
                        nc.vector.tensor_scalar_add(
                            o_half[:, cf, :], ps[:], bias_col[:, cf:cf + 1]
                        )
                    us.append(u)
                return us

            def projB_units(b):
                """8 closures: projection over oT head-pairs 4-7, combined
                with o_half, stored; carried into the NEXT batch."""
                us = []
                for cf in range(CT):
                    def u(cf=cf, b=b):
                        ps = psS.tile([128, N], F32, tag="s",
                                      name=f"pb_{b}_{cf}")
                        for ct in range(4, CT):
                            for j in (0, 512):
                                nc.tensor.matmul(
                                    ps[:, j:j + 512],
                                    wp_sb[:, ct, cf * 128:(cf + 1) * 128],
                                    oT[:, ct, j:j + 512],
                                    start=(ct == 4), stop=(ct == CT - 1),
                                )
                        o_sb = osb.tile([128, N], F32, tag="o",
                                        name=f"osb_{b}_{cf}")
                        nc.vector.tensor_add(
                            o_sb[:], ps[:], o_half[:, cf, :]
                        )
                        nc.sync.dma_start(
                            out=outT_d[b, cf * 128:(cf + 1) * 128, :],
                            in_=o_sb[:],
                        )
                    us.append(u)
                return us

            def tile_unit(b, nt):
                """x tile -> bf16 -> x^T (sync-HW-queue DMA, ACT cast, PE
                transposes).  Returned as a closure so the NEXT batch's
                tiles can run as lazy filler in THIS batch's tail pairs.
                (XBAR DMA transpose was measured: 208B packets, 855us total
                -- the PE path is far faster for 128x128 tiles.)"""
                def u():
                    xl = xload.tile([128, C], F32, tag="xl",
                                    name=f"xl_{b}_{nt}")
                    nc.sync.dma_start(
                        out=xl[:], in_=x_d[b, nt * 128:(nt + 1) * 128, :]
                    )
                    xbf = xbfp.tile([128, C], BF16, tag="xbf",
                                    name=f"xbf_{b}_{nt}")
                    nc.vector.tensor_copy(xbf[:], xl[:])
                    ps_t = psG.tile([128, CT, 128], BF16, tag="g",
                                    name=f"pst_{b}_{nt}")
                    for ct in range(CT):
                        nc.tensor.transpose(
                            ps_t[:, ct, :],
                            xbf[:, ct * 128:(ct + 1) * 128],
                            ident_bf[:],
                        )
                    nc.vector.tensor_copy(
                        xT[:, :, nt * 128:(nt + 1) * 128], ps_t[:]
                    )
                return u

            def emit_batch(b, carry, prefolded, fold_next):
                """Emit one batch.  `carry` = projB closures of the previous
                batch (drained in this preamble).  `prefolded`: this batch's
                x tiles + qk pairs 0/1 already ran in the previous batch's
                tail.  `fold_next`: (tiles, qks) closures of the NEXT batch
                to spread into this batch's pairs 6-7."""
                units = deque(carry)

                def drain(k=1):
                    for _ in range(k):
                        if units:
                            units.popleft()()

                if b > 0:
                    # prefix staging for this batch (casting gpsimd DMAs;
                    # the gpsimd engine reaches these while the previous
                    # attention still runs -> prefetch)
                    nc.gpsimd.dma_start(out=pkl[:], in_=pk_d[b])

                vb0 = v_units(b, 0)
                if not prefolded:
                    # the qk GEMM for token half jh only reads xT columns
                    # jh*512..+512 (= x tiles 4jh..4jh+3), so half the qk
                    # and v work starts after only FOUR tiles are
                    # transposed -- the PE chews on it while tiles 4-7
                    # stream in.
                    qk0 = qk_units(b, 0)   # [k-jh0, k-jh1, q-jh0, q-jh1]
                    qk1 = qk_units(b, 1)
                    for nt in range(4):
                        tile_unit(b, nt)()
                        drain(1)
                    for u in (qk0[0], qk0[2], qk1[0], qk1[2]):
                        u()
                        drain(1)
                    for nt in range(4):
                        vb0[nt]()
                        if nt < 2:
                            tile_unit(b, 4 + nt)()
                        drain(1)
                    tile_unit(b, 6)()
                    tile_unit(b, 7)()
                else:
                    qk0 = qk1 = None
                # prefix: pk^T into kPre cols 0:16
                ps_pk = psG.tile([128, CT, P], BF16, tag="g",
                                 name=f"pspk_{b}")
                for ct in range(CT):
                    nc.tensor.transpose(
                        ps_pk[:, ct, :],
                        pkl[:, ct * 128:(ct + 1) * 128],
                        ident_bf[0:P, 0:P],
                    )
                nc.vector.tensor_copy(kPre[:, :, 0:P], ps_pk[:])
                if b > 0:
                    _pv_load(b)
                if not prefolded:
                    for u in (qk0[1], qk0[3], qk1[1], qk1[3]):
                        u()
                        drain(1)
                    for nt in range(4, NT):
                        vb0[nt]()
                        drain(1)
                else:
                    for nt in range(NT):
                        vb0[nt]()
                        drain(1)
                drain(len(units))  # force out any remaining carry

                def prefix_group(g):
                    """Packed prefix scores for heads 4g..4g+3 (pairs 2g,
                    2g+1): head h's 16 prefix keys land on psum rows
                    32*(h%4)..+32 (stationary is 32 wide, cols 16:32 zero),
                    so ONE exp serves 4 heads.  MMs are ordered row-half-
                    major so only verified-safe masked||masked overlap can
                    occur."""
                    ps_pre = psS.tile([128, N], F32, tag="s",
                                      name=f"pre_{b}_{g}")
                    for hh in (0, 1):
                        base = hh * 64
                        for hg in (hh, hh + 2):
                            h = 4 * g + hg
                            p = h // 2
                            for j in (0, 512):
                                nc.tensor.matmul(
                                    ps_pre[32 * hg:32 * hg + 32, j:j + 512],
                                    kPre[base:base + D, p, :],
                                    qT[base:base + D, p % 4, j:j + 512],
                                    start=True, stop=True,
                                    tile_position=(base, 32 * hg),
                                )
                    e_pre = epre_pool.tile([128, N], BF16, tag="ep",
                                           name=f"ep_{b}_{g}")
                    nc.scalar.activation(e_pre[:], ps_pre[:], AF.Exp,
                                         scale=SCALE)
                    return e_pre

                e_pre = prefix_group(0)

                # ---- per-head attention, gemm pipeline in the slots.
                # urgent = next-next pair's q/k (deadline: pair p+1 end);
                # lazy, in deadline order: v block 1 (pair 4), own projA
                # (before projB in the next preamble), next batch's x tiles
                # and qk pairs 0/1 (its preamble) ----
                urgent = deque()
                lazy = deque()
                for p in range(HPAIRS):
                    if p + 2 < HPAIRS:
                        urgent.extend(qk_units(b, p + 2))
                    if p == 0:
                        lazy.extend(v_units(b, 1))
                    if p == 4:
                        lazy.extend(projA_units(b))
                    if p == 6 and fold_next is not None:
                        lazy.extend(fold_next[0])
                    if p == 7 and fold_next is not None:
                        lazy.extend(fold_next[1])
                    if p >= 2 and p % 2 == 0:
                        e_pre = prefix_group(p // 2)
                    slot = 0
                    for hh in range(2):
                        base = hh * 64
                        h = 2 * p + hh
                        ps_av = psAV.tile([128, N], F32, tag="av",
                                          name=f"av_{b}_{h}")
                        # prefix contribution from the shared packed exp
                        for j in (0, 512):
                            nc.tensor.matmul(
                                ps_av[:, j:j + 512],
                                v_ext[:, 0, h, :],
                                e_pre[:, j:j + 512],
                                start=True, stop=False,
                            )
                        for mt in range(1, MT):
                            ps_s = psS.tile([128, N], F32, tag="s",
                                            name=f"s_{b}_{h}_{mt}")
                            for j in (0, 512):
                                nc.tensor.matmul(
                                    ps_s[:, j:j + 512],
                                    kT[base:base + D, p % 4,
                                       (mt - 1) * 128:mt * 128],
                                    qT[base:base + D, p % 4, j:j + 512],
                                    start=True, stop=True,
                                )
                            eT = e_pool.tile([128, N], BF16, tag="e",
                                             name=f"e_{b}_{h}_{mt}")
                            nc.scalar.activation(eT[:], ps_s[:], AF.Exp,
                                                 scale=SCALE)
                            # gemm/proj filler BETWEEN exp and av: the PE
                            # would otherwise idle waiting for the exp (and,
                            # at mt==1, for the previous head's psum release)
                            slot += 1
                            if urgent and (mt in (1, 5)
                                           or len(urgent) >= 18 - slot):
                                urgent.popleft()()
                            elif lazy and (mt in (3, 7)
                                           or (p >= 4 and not urgent)):
                                lazy.popleft()()
                            for j in (0, 512):
                                nc.tensor.matmul(
                                    ps_av[:, j:j + 512],
                                    v_ext[:, mt, h, :],
                                    eT[:, j:j + 512],
                                    start=False, stop=(mt == MT - 1),
                                )
                        # normalize: out = unnorm * exp(-ln(denom)).
                        # (custom-DVE reciprocal_approx is unsupported by this
                        # walrus; iterative DVE reciprocal costs 6.5us.)
                        # The numerator is copied to SBUF so the psum
                        # accumulator is released after ~1.1us (copy || ln)
                        # instead of after the full ln->exp->mul chain.
                        num_sb = stg.tile([64, N], F32, tag="st",
                                          name=f"st_{b}_{h}")
                        nc.vector.tensor_copy(num_sb[:], ps_av[0:64, :])
                        lnd = rb_pool.tile([64, N], F32, tag="ln",
                                           name=f"ln_{b}_{h}")
                        nc.scalar.activation(lnd[:], ps_av[64:128, :], AF.Ln)
                        rb = rb_pool.tile([64, N], F32, tag="rb",
                                          name=f"rb_{b}_{h}")
                        nc.scalar.activation(rb[:], lnd[:], AF.Exp,
                                             scale=-1.0)
                        nc.vector.tensor_mul(
                            oT[base:base + D, p, :], num_sb[:], rb[:]
                        )
                    if p == HPAIRS - 1:
                        # end of batch: flush stragglers
                        while urgent:
                            urgent.popleft()()
                        while lazy:
                            lazy.popleft()()

                return projB_units(b)

            carry = []
            prefolded = False
            total = repeat * B_PC
            for i in range(total):
                b = i % B_PC
                fold_next = None
                if FOLD_NEXT and i + 1 < total:
                    nb = (i + 1) % B_PC
                    fold_next = (
                        [tile_unit(nb, nt) for nt in range(NT)],
                        qk_units(nb, 0) + qk_units(nb, 1),
                    )
                carry = emit_batch(b, carry, prefolded, fold_next)
                prefolded = fold_next is not None
            for u in carry:
                u()

    return nc


_NC_CACHE = {}


def _get_nc(repeat: int = 1) -> bass.Bass:
    key = f"nc{repeat}"
    if key not in _NC_CACHE:
        _NC_CACHE[key] = build_nc(repeat)
    return _NC_CACHE[key]


def _make_runner(nc):
    """Compile the SPMD kernel ONCE into a reusable callable.

    Mirrors bass2jax.run_bass_via_pjrt's multi-core branch, but without
    output-buffer donation so the compiled function + device-resident
    inputs can be invoked repeatedly (for wall-clock benchmarking and to
    avoid recompiles on every kernel() call).
    """
    import jax
    from jax.experimental.shard_map import shard_map
    from jax.sharding import Mesh, PartitionSpec
    from concourse import bass2jax
    from concourse.bass2jax import _bass_exec_p, partition_id_tensor

    bass2jax.install_neuronx_cc_hook()

    partition_name = (
        nc.partition_id_tensor.name if nc.partition_id_tensor else None
    )
    in_names, out_names, out_avals, zero_outs = [], [], [], []
    for alloc in nc.m.functions[0].allocations:
        if not isinstance(alloc, mybir.MemoryLocationSet):
            continue
        name = alloc.memorylocations[0].name
        if alloc.kind == "ExternalInput":
            if name != partition_name:
                in_names.append(name)
        elif alloc.kind == "ExternalOutput":
            shape = tuple(alloc.tensor_shape)
            dtype = mybir.dt.np(alloc.dtype)
            out_names.append(name)
            out_avals.append(jax.core.ShapedArray(shape, dtype))
            zero_outs.append(np.zeros(shape, dtype))
    n_params = len(in_names)
    all_in_names = list(in_names) + list(out_names)
    if partition_name is not None:
        all_in_names.append(partition_name)

    def _body(*args):
        operands = list(args)
        if partition_name is not None:
            operands.append(partition_id_tensor())
        outs = _bass_exec_p.bind(
            *operands,
            out_avals=tuple(out_avals),
            in_names=tuple(all_in_names),
            out_names=tuple(out_names),
            lowering_input_output_aliases=(),
            sim_require_finite=True,
            sim_require_nnan=True,
            nc=nc,
        )
        return tuple(outs)

    devices = jax.devices()[:N_CORES]
    mesh = Mesh(np.asarray(devices), ("core",))
    n_outs = len(out_avals)
    in_specs = (PartitionSpec("core"),) * (n_params + n_outs)
    out_specs = (PartitionSpec("core"),) * n_outs
    sharded = jax.jit(
        shard_map(_body, mesh=mesh, in_specs=in_specs,
                  out_specs=out_specs, check_rep=False),
        keep_unused=True,
    )

    concat_zeros = [
        np.zeros((N_CORES * z.shape[0], *z.shape[1:]), z.dtype)
        for z in zero_outs
    ]

    state = {"dev_zeros": None}

    def runner(in_maps):
        per_core = [
            [np.asarray(m[name]) for name in in_names] for m in in_maps
        ]
        concat_in = [
            np.concatenate([per_core[c][i] for c in range(N_CORES)], axis=0)
            for i in range(n_params)
        ]
        if state["dev_zeros"] is None:
            state["dev_zeros"] = [jax.device_put(z) for z in concat_zeros]
        out_arrs = sharded(*concat_in, *state["dev_zeros"])
        return [
            {
                name: np.asarray(out_arrs[i]).reshape(
                    N_CORES, *out_avals[i].shape
                )[c]
                for i, name in enumerate(out_names)
            }
            for c in range(N_CORES)
        ]

    def runner_dev(dev_args):
        """dev_args: device-resident concat inputs; returns device outputs."""
        return sharded(*dev_args, *state["dev_zeros"])

    def make_dev_args(in_maps):
        per_core = [
            [np.asarray(m[name]) for name in in_names] for m in in_maps
        ]
        concat_in = [
            np.concatenate([per_core[c][i] for c in range(N_CORES)], axis=0)
            for i in range(n_params)
        ]
        if state["dev_zeros"] is None:
            state["dev_zeros"] = [jax.device_put(z) for z in concat_zeros]
        return [jax.device_put(a) for a in concat_in]

    return runner, runner_dev, make_dev_args


def _get_runner(repeat: int = 1):
    key = f"runner{repeat}"
    if key not in _NC_CACHE:
        _NC_CACHE[key] = _make_runner(_get_nc(repeat))
    return _NC_CACHE[key]


def _make_in_maps(x, pk, pv, w_qkv, w_proj, b_proj):
    x = np.ascontiguousarray(np.asarray(x, dtype=np.float32))
    pk = np.ascontiguousarray(np.asarray(pk, dtype=np.float32))
    pv = np.ascontiguousarray(np.asarray(pv, dtype=np.float32))
    w_qkv = np.ascontiguousarray(np.asarray(w_qkv, dtype=np.float32))
    w_proj = np.ascontiguousarray(np.asarray(w_proj, dtype=np.float32))
    b_proj = np.ascontiguousarray(np.asarray(b_proj, dtype=np.float32))
    in_maps = []
    for c in range(N_CORES):
        sl = slice(c * B_PC, (c + 1) * B_PC)
        in_maps.append({
            "x": x[sl], "pk": pk[sl], "pv": pv[sl],
            "w_qkv": w_qkv, "w_proj": w_proj, "b_proj": b_proj,
        })
    return in_maps


def run(x, pk, pv, w_qkv, w_proj, b_proj, trace=False, **trace_kwargs):
    """Run the SPMD kernel; returns (output [B,N,C], results).

    With trace=True, routes through run_bass_kernel_spmd so the returned
    results object carries .exec_time_ns / .profile_json.
    """
    in_maps = _make_in_maps(x, pk, pv, w_qkv, w_proj, b_proj)
    if trace:
        res = run_bass_kernel_spmd(
            _get_nc(), in_maps, list(range(N_CORES)), trace=True,
            **trace_kwargs,
        )
        results = res.results
        out = np.empty((B, N, C), dtype=np.float32)
        for c in range(N_CORES):
            outT = results[c]["outT"]          # [B_PC, C, N]
            out[c * B_PC:(c + 1) * B_PC] = outT.transpose(0, 2, 1)
        return out, res
    runner, _, _ = _get_runner()
    results = runner(in_maps)
    out = np.empty((B, N, C), dtype=np.float32)
    for c in range(N_CORES):
        outT = results[c]["outT"]              # [B_PC, C, N]
        out[c * B_PC:(c + 1) * B_PC] = outT.transpose(0, 2, 1)
    return out, results


def kernel(x, pk, pv, w_qkv, w_proj, b_proj) -> np.ndarray:
    out, _ = run(x, pk, pv, w_qkv, w_proj, b_proj)
    return out


def benchmark(x, pk, pv, w_qkv, w_proj, b_proj, iters=20, warmup=3, repeat=1):
    """Median wall-clock per executed call with device-resident inputs."""
    import time
    import jax
    _, runner_dev, make_dev_args = _get_runner(repeat)
    in_maps = _make_in_maps(x, pk, pv, w_qkv, w_proj, b_proj)
    dev_args = make_dev_args(in_maps)
    for _ in range(warmup):
        outs = runner_dev(dev_args)
        jax.block_until_ready(outs)
    ts = []
    for _ in range(iters):
        t0 = time.perf_counter()
        outs = runner_dev(dev_args)
        jax.block_until_ready(outs)
        ts.append(time.perf_counter() - t0)
    ts.sort()
    return {
        "median_s": ts[len(ts) // 2],
        "min_s": ts[0],
        "all_s": ts,
    }



# revision 33
# speedup vs baseline: 1.0124x; 1.0124x over previous
"""Trainium2 Bass kernel for prefix-KV multi-head attention (v2).

Reference computation (per batch):
    qkv = x @ w_qkv -> q,k,v heads; k/v get a 16-token prefix (pk, pv)
    attn = softmax(q @ k^T * D^-0.5); out = (attn @ v) @ w_proj + b_proj

Sharding: data-parallel over B across 8 NeuronCores (2 batches per core).

Design (vs the v1 baseline, 700us -> 608us):
  - weights loaded to SBUF once per core (bf16), reused by both batches
  - q^T kept in SBUF (no DRAM spill)
  - v computed in NATURAL [token, feature] layout via x^T-stationary GEMM
    (moving = w_v columns), eliminating all per-head v transposes
  - x^T built with bf16 PE transposes (2x faster than fp32)
  - attention runs per HEAD (not head-pair): PSUM = scores 2x2 banks
    (double buffered) + av accumulator 2 banks + gemm scratch 2x1 banks
    = 8 banks exactly
  - q/k/v GEMM chunks for pair p+1 and proj passes of the previous batch
    are software-pipelined into the attention mt-loop slots, so the PE
    stays busy while ACT computes exp()
  - softmax 1/denominator via exp(-ln(d)) on ACT (this walrus lacks the
    custom-DVE approx ops; iterative DVE reciprocal costs 6.5us); a DVE
    copy of the numerator releases the av psum accumulator early
  - ones-columns packed next to v in v_ext give the softmax denominator
    for free inside the attention@v matmul (rows 64:128 of the psum)

Explored and rejected (all measured on HW): fp8 (2e-2 tolerance
exceeded: random-sign GEMM error stays ~5.7% relative regardless of N);
PE tile-packing of the K=64 score matmuls (verified ~1.9x overlap on
alternating-row-half pairs via microbenchmark, but a full-array matmul
issued behind a packed pair corrupts the array unless sync-guarded, and
guarded variants measured 641-759us vs 608us -- see kernel_v5.py /
kernel_v4_packed.py); normalize multiply on the Pool engine (Pool
tensor ops ~3x slower than modeled: 742us); deferring the normalize mul
by one head (624us); 1024-col moving matmuls (hardware ISA caps moving
at 512).

This file is self-contained: it monkeypatches two workarounds for the
walrus build in this container (1-sync-wait-per-instruction cap).
"""

import json
import os
import sys
from collections import deque

for _p in ("/opt/trn_rl_repo", os.path.expanduser("~/.axon_site/_ro/trn_rl_repo")):
    if os.path.isdir(_p) and _p not in sys.path:
        sys.path.insert(0, _p)

import numpy as np

import concourse.bass as bass
import concourse.tile as tile
from concourse import mybir
from concourse.bass_utils import run_bass_kernel_spmd
from concourse.vector_clock import ScopedClock
from concourse.masks import make_identity

F32 = mybir.dt.float32
BF16 = mybir.dt.bfloat16
AF = mybir.ActivationFunctionType

# ---------------------------------------------------------------------------
# Workaround: this container's walrus supports at most ONE sync wait per
# instruction.  (a) split the TileContext-exit drain's waits onto single-wait
# NOPs; (b) at BIR-JSON serialization time, hoist extra waits from any
# instruction onto same-engine NOPs placed immediately before it.
# ---------------------------------------------------------------------------

def _patched_drain_and_barrier(self, tick_clock, wait_clock):
    drain_inst = self.nc.sync.drain()
    wait_clock.add_sem_waits(
        drain_inst.ins, ScopedClock({None: tick_clock.global_clock})
    )
    si = drain_inst.ins.sync_info
    waits = list(si.on_wait) if si is not None and si.on_wait else []
    if len(waits) > 1:
        si.on_wait = waits[:1]
        for w in waits[1:]:
            nop = self.nc.sync.nop(hint="drain_wait_split", nofuse=True)
            nsi = nop.ins.sync_info
            if nsi is None:
                nop.ins.sync_info = mybir.SyncInfo(on_wait=[w], on_update=[])
            else:
                nsi.on_wait = list(nsi.on_wait or []) + [w]
    self.nc.all_engine_barrier()
    assert self.sems is not None
    popped = self.nc._tile_sem_poison_stack.pop()
    assert popped is self._sem_poison
    self.nc.clear_and_free_semaphores(list(self.sems.allocated().values()))
    self.nc.all_engine_barrier()


tile.TileContext._drain_and_barrier = _patched_drain_and_barrier


def _split_multi_waits(bir):
    for fn in bir["functions"]:
        for bb in fn["blocks"]:
            new_insts = []
            for inst in bb["instructions"]:
                si = inst.get("sync_info")
                ow = (si or {}).get("on_wait") or []
                if len(ow) > 1:
                    for i, w in enumerate(ow[:-1]):
                        new_insts.append({
                            "debug": inst.get("debug", 0),
                            "engine": inst["engine"],
                            "ins": [], "outs": [],
                            "name": f"{inst['name']}.wsplit{i}",
                            "opcode": "NoOp",
                            "sync_info": {"on_wait": [w], "on_update": []},
                        })
                    si["on_wait"] = [ow[-1]]
                new_insts.append(inst)
            bb["instructions"] = new_insts
    return bir


_orig_to_json_bytes = bass.Bass.to_json_bytes


def _patched_to_json_bytes(self):
    d = json.loads(_orig_to_json_bytes(self))
    _split_multi_waits(d)
    return json.dumps(d).encode()


bass.Bass.to_json_bytes = _patched_to_json_bytes

# ---------------------------------------------------------------------------
# Problem constants (hardcoded per the task contract)
# ---------------------------------------------------------------------------

B, N, C, H, P = 16, 1024, 1024, 16, 16
D = C // H                      # 64
SCALE = float(D) ** -0.5        # 0.125
N_CORES = 8
B_PC = B // N_CORES             # 2 batches per core
NT = N // 128                   # 8 token tiles
CT = C // 128                   # 8 feature tiles
MT = NT + 1                     # 9 m-tiles: tile 0 = prefix (16 valid rows)
HPAIRS = H // 2                 # 8 head pairs
FOLD_NEXT = True


def build_nc(repeat: int = 1) -> bass.Bass:
    nc = bass.Bass()

    x_d = nc.declare_dram_parameter("x", [B_PC, N, C], F32, isOutput=False)
    pk_d = nc.declare_dram_parameter("pk", [B_PC, P, C], F32, isOutput=False)
    pv_d = nc.declare_dram_parameter("pv", [B_PC, P, C], F32, isOutput=False)
    wqkv_d = nc.declare_dram_parameter("w_qkv", [C, 3 * C], F32, isOutput=False)
    wproj_d = nc.declare_dram_parameter("w_proj", [C, C], F32, isOutput=False)
    bias_d = nc.declare_dram_parameter("b_proj", [C], F32, isOutput=False)
    # output is stored TRANSPOSED per batch: [C, N]; host transposes back
    outT_d = nc.declare_dram_parameter("outT", [B_PC, C, N], F32, isOutput=True)

    with tile.TileContext(nc) as tc:
        with tc.tile_pool(name="cons", bufs=1) as cons, \
             tc.tile_pool(name="eP", bufs=3) as e_pool, \
             tc.tile_pool(name="ePre", bufs=1) as epre_pool, \
             tc.tile_pool(name="stg", bufs=1) as stg, \
             tc.tile_pool(name="rbp", bufs=1) as rb_pool, \
             tc.tile_pool(name="xload", bufs=2) as xload, \
             tc.tile_pool(name="xbf", bufs=2) as xbfp, \
             tc.tile_pool(name="osb", bufs=2) as osb, \
             tc.tile_pool(name="psS", bufs=2, space="PSUM") as psS, \
             tc.tile_pool(name="psAV", bufs=1, space="PSUM") as psAV, \
             tc.tile_pool(name="psG", bufs=2, space="PSUM") as psG:

            # ---------------- one-time setup ----------------
            ident_bf = cons.tile([128, 128], BF16, tag="idb")
            make_identity(nc, ident_bf[:])
            # PE warm-up burst: ~3.5us of throwaway matmuls releases the
            # HAM clock-gate (K=4/8 -> 8/8) before the real work arrives,
            # so the preamble transposes/GEMMs run at 2.4 GHz not 1.2.
            warm_ps = psG.tile([128, 128], F32, tag="g", name="warmup")
            for _w in range(32):
                nc.tensor.matmul(
                    warm_ps[:], ident_bf[:], ident_bf[:],
                    start=(_w == 0), stop=(_w == 31),
                )
            # bias in per-partition layout: bias_col[p, cf] = b_proj[cf*128+p]
            bias_col = cons.tile([128, CT], F32, tag="bias")
            nc.sync.dma_start(
                out=bias_col[:],
                in_=bias_d[:].rearrange("(a b) -> b a", b=128),
            )
            # prefix-k staging (bf16 via casting gpsimd DMA)
            pkl = cons.tile([P, C], BF16, tag="pkl")

            # persistent activations (reused across batches; Tile tracks
            # read/write hazards on AP ranges).  qT/kT hold THREE head
            # pairs (slot p%3): pair p+2 is produced by pipelined fillers
            # while pair p's attention reads its slot; the extra slot lets
            # the packed-prefix exp (4 heads = 2 pairs per ACTIVATE) see
            # both of its pairs' q at group start.
            xT = cons.tile([128, CT, N], BF16, tag="xT")
            kT = cons.tile([128, 4, N], BF16, tag="kT")
            qT = cons.tile([128, 4, N], BF16, tag="qT")
            # prefix keys, all pairs: cols 0:16 = pk^T, 16:32 zero so the
            # packed 32-row score stripes come out 0 on rows 16:32 ->
            # exp = 1, harmless because the matching v_ext rows are zero
            kPre = cons.tile([128, HPAIRS, 32], BF16, tag="kPre")
            nc.vector.memset(kPre[:, :, P:32], 0.0)
            oT = cons.tile([128, CT, N], BF16, tag="oT")
            # first-half (head pairs 0-3) projection partials, bf16; the A
            # pass runs as lazy filler inside the SAME batch's pairs 4+,
            # the B pass (pairs 4-7 + combine + store) carries to the next
            # batch's preamble
            o_half = cons.tile([128, CT, N], BF16, tag="oh")
            # v_ext[m, mt, h, 0:64] = v values; [.., 64:128] = ones columns
            # (denominator trick). m-tile 0 = prefix, PACKED: head h's 16
            # pv rows live at partitions 32*(h%4)..+16 (matching its stripe
            # in the packed prefix-score psum); all other rows stay ZERO so
            # the other heads' e values in the shared e_pre tile contribute
            # nothing to this head's av or denominator.
            v_ext = cons.tile([128, MT, H, 128], BF16, tag="vx")
            nc.vector.memset(v_ext[:, :, :, 64:128], 1.0)
            nc.vector.memset(v_ext[:, 0, :, :], 0.0)
            for a in range(4):
                nc.vector.memset(
                    v_ext[32 * a:32 * a + P, 0, a::4, 64:128], 1.0
                )

            # weights, bf16, resident for the whole kernel, on the gpsimd
            # sw-DGE queue (the only one that casts).  512-col chunks keep
            # the write packets at 1KB (128-col chunks made 256B packets and
            # left the queue packet-rate-bound for ~60us).  x rides the
            # separate sync HW queue concurrently.
            wq_sb = cons.tile([128, CT, C], BF16, tag="wq")
            wk_sb = cons.tile([128, CT, C], BF16, tag="wk")
            wv_sb = cons.tile([128, CT, C], BF16, tag="wv")
            wp_sb = cons.tile([128, CT, C], BF16, tag="wp")

            def _wload(dst, base, lo, hi):
                nc.gpsimd.dma_start(
                    out=dst[:, :, lo:hi],
                    in_=wqkv_d[:, base + lo:base + hi].rearrange(
                        "(ct p) f -> p ct f", p=128),
                )

            def _pv_load(b):
                pvr = pv_d[b].rearrange("t (h d) -> t h d", d=64)
                for a in range(4):
                    nc.gpsimd.dma_start(
                        out=v_ext[32 * a:32 * a + P, 0, a::4, 0:64],
                        in_=pvr[:, a::4, :],
                    )

            nc.gpsimd.dma_start(out=pkl[:], in_=pk_d[0])
            _wload(wk_sb, C, 0, 128)                  # k pair 0
            _wload(wq_sb, 0, 0, 128)                  # q pair 0
            _wload(wq_sb, 0, 128, 256)                # q pair 1
            _wload(wk_sb, C, 128, 256)                # k pair 1
            _wload(wv_sb, 2 * C, 0, 512)              # v block 0
            _pv_load(0)                               # prefix v, batch 0
            _wload(wv_sb, 2 * C, 512, 1024)           # v block 1
            _wload(wk_sb, C, 256, 640)
            _wload(wq_sb, 0, 256, 640)
            _wload(wk_sb, C, 640, 1024)
            _wload(wq_sb, 0, 640, 1024)
            nc.gpsimd.dma_start(
                out=wp_sb[:],
                in_=wproj_d[:].rearrange("(ct p) f -> p ct f", p=128),
            )

            # ---------------- per-batch work units ----------------

            def qk_units(b, p):
                """4 closures: q and k GEMMs for head pair p, split in two
                512-column halves each. Each accumulates 8 c-tiles into a
                [128,512] psum and copies (cast bf16) into qT/kT."""
                us = []
                for which in ("k", "q"):
                    for jh in range(2):
                        def u(which=which, p=p, jh=jh, b=b):
                            w_sb = wk_sb if which == "k" else wq_sb
                            ps = psG.tile([128, 512], F32, tag="g",
                                          name=f"g{which}_{b}_{p}_{jh}")
                            for ct in range(CT):
                                nc.tensor.matmul(
                                    ps[:],
                                    w_sb[:, ct, p * 128:(p + 1) * 128],
                                    xT[:, ct, jh * 512:(jh + 1) * 512],
                                    start=(ct == 0), stop=(ct == CT - 1),
                                )
                            if which == "k":
                                nc.vector.tensor_copy(
                                    kT[:, p % 4, jh * 512:(jh + 1) * 512],
                                    ps[:],
                                )
                            else:
                                nc.vector.tensor_copy(
                                    qT[:, p % 4, jh * 512:(jh + 1) * 512],
                                    ps[:],
                                )
                        us.append(u)
                return us

            def v_units(b, bk):
                """8 closures: v GEMM for pair block bk (4 pairs = 512 v
                columns), one per token tile. x^T tile is stationary, w_v
                columns are moving -> v lands in NATURAL [token, feature]
                layout, no transpose needed."""
                us = []
                for nt in range(NT):
                    def u(nt=nt, bk=bk, b=b):
                        ps = psG.tile([128, 512], F32, tag="g",
                                      name=f"gv_{b}_{bk}_{nt}")
                        for ct in range(CT):
                            nc.tensor.matmul(
                                ps[:],
                                xT[:, ct, nt * 128:(nt + 1) * 128],
                                wv_sb[:, ct, bk * 512:(bk + 1) * 512],
                                start=(ct == 0), stop=(ct == CT - 1),
                            )
                        nc.vector.tensor_copy(
                            v_ext[:, nt + 1, 8 * bk:8 * (bk + 1), 0:64],
                            ps[:].rearrange("p (h d) -> p h d", d=64),
                        )
                    us.append(u)
                return us

            def projA_units(b):
                """8 closures: projection over oT head-pairs 0-3 (+bias)
                into bf16 o_half.  Ready once pair 3 is normalized, so they
                fill THIS batch's pair 4+ attention slots."""
                us = []
                for cf in range(CT):
                    def u(cf=cf, b=b):
                        ps = psS.tile([128, N], F32, tag="s",
                                      name=f"pa_{b}_{cf}")
                        for ct in range(4):
                            for j in (0, 512):
                                nc.tensor.matmul(
                                    ps[:, j:j + 512],
                                    wp_sb[:, ct, cf * 128:(cf + 1) * 128],
                                    oT[:, ct, j:j + 512],
                                    start=(ct == 0), stop=(ct == 3),
                                )
                        nc.vector.tensor_scalar_add(
                            o_half[:, cf, :], ps[:], bias_col[:, cf:cf + 1]
                        )
                    us.append(u)
                return us

            def projB_units(b):
                """8 closures: projection over oT head-pairs 4-7, combined
                with o_half, stored; carried into the NEXT batch."""
                us = []
                for cf in range(CT):
                    def u(cf=cf, b=b):
                        ps = psS.tile([128, N], F32, tag="s",
                                      name=f"pb_{b}_{cf}")
                        for ct in range(4, CT):
                            for j in (0, 512):
                                nc.tensor.matmul(
                                    ps[:, j:j + 512],
                                    wp_sb[:, ct, cf * 128:(cf + 1) * 128],
                                    oT[:, ct, j:j + 512],
                                    start=(ct == 4), stop=(ct == CT - 1),
                                )
                        o_sb = osb.tile([128, N], F32, tag="o",
                                        name=f"osb_{b}_{cf}")
                        nc.vector.tensor_add(
                            o_sb[:], ps[:], o_half[:, cf, :]
                        )
                        nc.sync.dma_start(
                            out=outT_d[b, cf * 128:(cf + 1) * 128, :],
                            in_=o_sb[:],
                        )
                    us.append(u)
                return us

            def tile_unit(b, nt):
                """x tile -> bf16 -> x^T (sync-HW-queue DMA, ACT cast, PE
                transposes).  Returned as a closure so the NEXT batch's
                tiles can run as lazy filler in THIS batch's tail pairs.
                (XBAR DMA transpose was measured: 208B packets, 855us total
                -- the PE path is far faster for 128x128 tiles.)"""
                def u():
                    xl = xload.tile([128, C], F32, tag="xl",
                                    name=f"xl_{b}_{nt}")
                    nc.sync.dma_start(
                        out=xl[:], in_=x_d[b, nt * 128:(nt + 1) * 128, :]
                    )
                    xbf = xbfp.tile([128, C], BF16, tag="xbf",
                                    name=f"xbf_{b}_{nt}")
                    nc.scalar.activation(xbf[:], xl[:], AF.Copy)
                    ps_t = psG.tile([128, CT, 128], BF16, tag="g",
                                    name=f"pst_{b}_{nt}")
                    for ct in range(CT):
                        nc.tensor.transpose(
                            ps_t[:, ct, :],
                            xbf[:, ct * 128:(ct + 1) * 128],
                            ident_bf[:],
                        )
                    nc.vector.tensor_copy(
                        xT[:, :, nt * 128:(nt + 1) * 128], ps_t[:]
                    )
                return u

            def emit_batch(b, carry, prefolded, fold_next):
                """Emit one batch.  `carry` = projB closures of the previous
                batch (drained in this preamble).  `prefolded`: this batch's
                x tiles + qk pairs 0/1 already ran in the previous batch's
                tail.  `fold_next`: (tiles, qks) closures of the NEXT batch
                to spread into this batch's pairs 6-7."""
                units = deque(carry)

                def drain(k=1):
                    for _ in range(k):
                        if units:
                            units.popleft()()

                if b > 0:
                    # prefix staging for this batch (casting gpsimd DMAs;
                    # the gpsimd engine reaches these while the previous
                    # attention still runs -> prefetch)
                    nc.gpsimd.dma_start(out=pkl[:], in_=pk_d[b])

                vb0 = v_units(b, 0)
                if not prefolded:
                    # the qk GEMM for token half jh only reads xT columns
                    # jh*512..+512 (= x tiles 4jh..4jh+3), so half the qk
                    # and v work starts after only FOUR tiles are
                    # transposed -- the PE chews on it while tiles 4-7
                    # stream in.
                    qk0 = qk_units(b, 0)   # [k-jh0, k-jh1, q-jh0, q-jh1]
                    qk1 = qk_units(b, 1)
                    for nt in range(4):
                        tile_unit(b, nt)()
                        drain(1)
                    for u in (qk0[0], qk0[2], qk1[0], qk1[2]):
                        u()
                        drain(1)
                    for nt in range(4):
                        vb0[nt]()
                        if nt < 2:
                            tile_unit(b, 4 + nt)()
                        drain(1)
                    tile_unit(b, 6)()
                    tile_unit(b, 7)()
                else:
                    qk0 = qk1 = None
                # prefix: pk^T into kPre cols 0:16
                ps_pk = psG.tile([128, CT, P], BF16, tag="g",
                                 name=f"pspk_{b}")
                for ct in range(CT):
                    nc.tensor.transpose(
                        ps_pk[:, ct, :],
                        pkl[:, ct * 128:(ct + 1) * 128],
                        ident_bf[0:P, 0:P],
                    )
                nc.vector.tensor_copy(kPre[:, :, 0:P], ps_pk[:])
                if b > 0:
                    _pv_load(b)
                if not prefolded:
                    for u in (qk0[1], qk0[3], qk1[1], qk1[3]):
                        u()
                        drain(1)
                    for nt in range(4, NT):
                        vb0[nt]()
                        drain(1)
                else:
                    for nt in range(NT):
                        vb0[nt]()
                        drain(1)
                drain(len(units))  # force out any remaining carry

                def prefix_group(g):
                    """Packed prefix scores for heads 4g..4g+3 (pairs 2g,
                    2g+1): head h's 16 prefix keys land on psum rows
                    32*(h%4)..+32 (stationary is 32 wide, cols 16:32 zero),
                    so ONE exp serves 4 heads.  MMs are ordered row-half-
                    major so only verified-safe masked||masked overlap can
                    occur."""
                    ps_pre = psS.tile([128, N], F32, tag="s",
                                      name=f"pre_{b}_{g}")
                    for hh in (0, 1):
                        base = hh * 64
                        for hg in (hh, hh + 2):
                            h = 4 * g + hg
                            p = h // 2
                            for j in (0, 512):
                                nc.tensor.matmul(
                                    ps_pre[32 * hg:32 * hg + 32, j:j + 512],
                                    kPre[base:base + D, p, :],
                                    qT[base:base + D, p % 4, j:j + 512],
                                    start=True, stop=True,
                                    tile_position=(base, 32 * hg),
                                )
                    e_pre = epre_pool.tile([128, N], BF16, tag="ep",
                                           name=f"ep_{b}_{g}")
                    nc.scalar.activation(e_pre[:], ps_pre[:], AF.Exp,
                                         scale=SCALE)
                    return e_pre

                e_pre = prefix_group(0)

                # ---- per-head attention, gemm pipeline in the slots.
                # urgent = next-next pair's q/k (deadline: pair p+1 end);
                # lazy, in deadline order: v block 1 (pair 4), own projA
                # (before projB in the next preamble), next batch's x tiles
                # and qk pairs 0/1 (its preamble) ----
                urgent = deque()
                lazy = deque()
                for p in range(HPAIRS):
                    if p + 2 < HPAIRS:
                        urgent.extend(qk_units(b, p + 2))
                    if p == 0:
                        lazy.extend(v_units(b, 1))
                    if p == 4:
                        lazy.extend(projA_units(b))
                    if p == 6 and fold_next is not None:
                        lazy.extend(fold_next[0])
                    if p == 7 and fold_next is not None:
                        lazy.extend(fold_next[1])
                    if p >= 2 and p % 2 == 0:
                        e_pre = prefix_group(p // 2)
                    slot = 0
                    for hh in range(2):
                        base = hh * 64
                        h = 2 * p + hh
                        ps_av = psAV.tile([128, N], F32, tag="av",
                                          name=f"av_{b}_{h}")
                        # prefix contribution from the shared packed exp
                        for j in (0, 512):
                            nc.tensor.matmul(
                                ps_av[:, j:j + 512],
                                v_ext[:, 0, h, :],
                                e_pre[:, j:j + 512],
                                start=True, stop=False,
                            )
                        for mt in range(1, MT):
                            ps_s = psS.tile([128, N], F32, tag="s",
                                            name=f"s_{b}_{h}_{mt}")
                            for j in (0, 512):
                                nc.tensor.matmul(
                                    ps_s[:, j:j + 512],
                                    kT[base:base + D, p % 4,
                                       (mt - 1) * 128:mt * 128],
                                    qT[base:base + D, p % 4, j:j + 512],
                                    start=True, stop=True,
                                )
                            eT = e_pool.tile([128, N], BF16, tag="e",
                                             name=f"e_{b}_{h}_{mt}")
                            nc.scalar.activation(eT[:], ps_s[:], AF.Exp,
                                                 scale=SCALE)
                            # gemm/proj filler BETWEEN exp and av: the PE
                            # would otherwise idle waiting for the exp (and,
                            # at mt==1, for the previous head's psum release)
                            slot += 1
                            if urgent and (mt in (1, 5)
                                           or len(urgent) >= 18 - slot):
                                urgent.popleft()()
                            elif lazy and (mt in (3, 7)
                                           or (p >= 4 and not urgent)):
                                lazy.popleft()()
                            for j in (0, 512):
                                nc.tensor.matmul(
                                    ps_av[:, j:j + 512],
                                    v_ext[:, mt, h, :],
                                    eT[:, j:j + 512],
                                    start=False, stop=(mt == MT - 1),
                                )
                        # normalize: out = unnorm * exp(-ln(denom)).
                        # (custom-DVE reciprocal_approx is unsupported by this
                        # walrus; iterative DVE reciprocal costs 6.5us.)
                        # The numerator is copied to SBUF so the psum
                        # accumulator is released after ~1.1us (copy || ln)
                        # instead of after the full ln->exp->mul chain.
                        num_sb = stg.tile([64, N], F32, tag="st",
                                          name=f"st_{b}_{h}")
                        nc.vector.tensor_copy(num_sb[:], ps_av[0:64, :])
                        lnd = rb_pool.tile([64, N], F32, tag="ln",
                                           name=f"ln_{b}_{h}")
                        nc.scalar.activation(lnd[:], ps_av[64:128, :], AF.Ln)
                        rb = rb_pool.tile([64, N], F32, tag="rb",
                                          name=f"rb_{b}_{h}")
                        nc.scalar.activation(rb[:], lnd[:], AF.Exp,
                                             scale=-1.0)
                        nc.vector.tensor_mul(
                            oT[base:base + D, p, :], num_sb[:], rb[:]
                        )
                    if p == HPAIRS - 1:
                        # end of batch: flush stragglers
                        while urgent:
                            urgent.popleft()()
                        while lazy:
                            lazy.popleft()()

                return projB_units(b)

            carry = []
            prefolded = False
            total = repeat * B_PC
            for i in range(total):
                b = i % B_PC
                fold_next = None
                if FOLD_NEXT and i + 1 < total:
                    nb = (i + 1) % B_PC
                    fold_next = (
                        [tile_unit(nb, nt) for nt in range(NT)],
                        qk_units(nb, 0) + qk_units(nb, 1),
                    )
                carry = emit_batch(b, carry, prefolded, fold_next)
                prefolded = fold_next is not None
            for u in carry:
                u()

    return nc


_NC_CACHE = {}


def _get_nc(repeat: int = 1) -> bass.Bass:
    key = f"nc{repeat}"
    if key not in _NC_CACHE:
        _NC_CACHE[key] = build_nc(repeat)
    return _NC_CACHE[key]


def _make_runner(nc):
    """Compile the SPMD kernel ONCE into a reusable callable.

    Mirrors bass2jax.run_bass_via_pjrt's multi-core branch, but without
    output-buffer donation so the compiled function + device-resident
    inputs can be invoked repeatedly (for wall-clock benchmarking and to
    avoid recompiles on every kernel() call).
    """
    import jax
    from jax.experimental.shard_map import shard_map
    from jax.sharding import Mesh, PartitionSpec
    from concourse import bass2jax
    from concourse.bass2jax import _bass_exec_p, partition_id_tensor

    bass2jax.install_neuronx_cc_hook()

    partition_name = (
        nc.partition_id_tensor.name if nc.partition_id_tensor else None
    )
    in_names, out_names, out_avals, zero_outs = [], [], [], []
    for alloc in nc.m.functions[0].allocations:
        if not isinstance(alloc, mybir.MemoryLocationSet):
            continue
        name = alloc.memorylocations[0].name
        if alloc.kind == "ExternalInput":
            if name != partition_name:
                in_names.append(name)
        elif alloc.kind == "ExternalOutput":
            shape = tuple(alloc.tensor_shape)
            dtype = mybir.dt.np(alloc.dtype)
            out_names.append(name)
            out_avals.append(jax.core.ShapedArray(shape, dtype))
            zero_outs.append(np.zeros(shape, dtype))
    n_params = len(in_names)
    all_in_names = list(in_names) + list(out_names)
    if partition_name is not None:
        all_in_names.append(partition_name)

    def _body(*args):
        operands = list(args)
        if partition_name is not None:
            operands.append(partition_id_tensor())
        outs = _bass_exec_p.bind(
            *operands,
            out_avals=tuple(out_avals),
            in_names=tuple(all_in_names),
            out_names=tuple(out_names),
            lowering_input_output_aliases=(),
            sim_require_finite=True,
            sim_require_nnan=True,
            nc=nc,
        )
        return tuple(outs)

    devices = jax.devices()[:N_CORES]
    mesh = Mesh(np.asarray(devices), ("core",))
    n_outs = len(out_avals)
    in_specs = (PartitionSpec("core"),) * (n_params + n_outs)
    out_specs = (PartitionSpec("core"),) * n_outs
    sharded = jax.jit(
        shard_map(_body, mesh=mesh, in_specs=in_specs,
                  out_specs=out_specs, check_rep=False),
        keep_unused=True,
    )

    concat_zeros = [
        np.zeros((N_CORES * z.shape[0], *z.shape[1:]), z.dtype)
        for z in zero_outs
    ]

    state = {"dev_zeros": None}

    def runner(in_maps):
        per_core = [
            [np.asarray(m[name]) for name in in_names] for m in in_maps
        ]
        concat_in = [
            np.concatenate([per_core[c][i] for c in range(N_CORES)], axis=0)
            for i in range(n_params)
        ]
        if state["dev_zeros"] is None:
            state["dev_zeros"] = [jax.device_put(z) for z in concat_zeros]
        out_arrs = sharded(*concat_in, *state["dev_zeros"])
        return [
            {
                name: np.asarray(out_arrs[i]).reshape(
                    N_CORES, *out_avals[i].shape
                )[c]
                for i, name in enumerate(out_names)
            }
            for c in range(N_CORES)
        ]

    def runner_dev(dev_args):
        """dev_args: device-resident concat inputs; returns device outputs."""
        return sharded(*dev_args, *state["dev_zeros"])

    def make_dev_args(in_maps):
        per_core = [
            [np.asarray(m[name]) for name in in_names] for m in in_maps
        ]
        concat_in = [
            np.concatenate([per_core[c][i] for c in range(N_CORES)], axis=0)
            for i in range(n_params)
        ]
        if state["dev_zeros"] is None:
            state["dev_zeros"] = [jax.device_put(z) for z in concat_zeros]
        return [jax.device_put(a) for a in concat_in]

    return runner, runner_dev, make_dev_args


def _get_runner(repeat: int = 1):
    key = f"runner{repeat}"
    if key not in _NC_CACHE:
        _NC_CACHE[key] = _make_runner(_get_nc(repeat))
    return _NC_CACHE[key]


def _make_in_maps(x, pk, pv, w_qkv, w_proj, b_proj):
    x = np.ascontiguousarray(np.asarray(x, dtype=np.float32))
    pk = np.ascontiguousarray(np.asarray(pk, dtype=np.float32))
    pv = np.ascontiguousarray(np.asarray(pv, dtype=np.float32))
    w_qkv = np.ascontiguousarray(np.asarray(w_qkv, dtype=np.float32))
    w_proj = np.ascontiguousarray(np.asarray(w_proj, dtype=np.float32))
    b_proj = np.ascontiguousarray(np.asarray(b_proj, dtype=np.float32))
    in_maps = []
    for c in range(N_CORES):
        sl = slice(c * B_PC, (c + 1) * B_PC)
        in_maps.append({
            "x": x[sl], "pk": pk[sl], "pv": pv[sl],
            "w_qkv": w_qkv, "w_proj": w_proj, "b_proj": b_proj,
        })
    return in_maps


def run(x, pk, pv, w_qkv, w_proj, b_proj, trace=False, **trace_kwargs):
    """Run the SPMD kernel; returns (output [B,N,C], results).

    With trace=True, routes through run_bass_kernel_spmd so the returned
    results object carries .exec_time_ns / .profile_json.
    """
    in_maps = _make_in_maps(x, pk, pv, w_qkv, w_proj, b_proj)
    if trace:
        res = run_bass_kernel_spmd(
            _get_nc(), in_maps, list(range(N_CORES)), trace=True,
            **trace_kwargs,
        )
        results = res.results
        out = np.empty((B, N, C), dtype=np.float32)
        for c in range(N_CORES):
            outT = results[c]["outT"]          # [B_PC, C, N]
            out[c * B_PC:(c + 1) * B_PC] = outT.transpose(0, 2, 1)
        return out, res
    runner, _, _ = _get_runner()
    results = runner(in_maps)
    out = np.empty((B, N, C), dtype=np.float32)
    for c in range(N_CORES):
        outT = results[c]["outT"]              # [B_PC, C, N]
        out[c * B_PC:(c + 1) * B_PC] = outT.transpose(0, 2, 1)
    return out, results


def kernel(x, pk, pv, w_qkv, w_proj, b_proj) -> np.ndarray:
    out, _ = run(x, pk, pv, w_qkv, w_proj, b_proj)
    return out


def benchmark(x, pk, pv, w_qkv, w_proj, b_proj, iters=20, warmup=3, repeat=1):
    """Median wall-clock per executed call with device-resident inputs."""
    import time
    import jax
    _, runner_dev, make_dev_args = _get_runner(repeat)
    in_maps = _make_in_maps(x, pk, pv, w_qkv, w_proj, b_proj)
    dev_args = make_dev_args(in_maps)
    for _ in range(warmup):
        outs = runner_dev(dev_args)
        jax.block_until_ready(outs)
    ts = []
    for _ in range(iters):
        t0 = time.perf_counter()
        outs = runner_dev(dev_args)
        jax.block_until_ready(outs)
        ts.append(time.perf_counter() - t0)
    ts.sort()
    return {
        "median_s": ts[len(ts) // 2],
        "min_s": ts[0],
        "all_s": ts,
    }



# revision 34
# speedup vs baseline: 1.0375x; 1.0248x over previous
"""Trainium2 Bass kernel for prefix-KV multi-head attention (v2).

Reference computation (per batch):
    qkv = x @ w_qkv -> q,k,v heads; k/v get a 16-token prefix (pk, pv)
    attn = softmax(q @ k^T * D^-0.5); out = (attn @ v) @ w_proj + b_proj

Sharding: data-parallel over B across 8 NeuronCores (2 batches per core).

Design (vs the v1 baseline, 700us -> 608us):
  - weights loaded to SBUF once per core (bf16), reused by both batches
  - q^T kept in SBUF (no DRAM spill)
  - v computed in NATURAL [token, feature] layout via x^T-stationary GEMM
    (moving = w_v columns), eliminating all per-head v transposes
  - x^T built with bf16 PE transposes (2x faster than fp32)
  - attention runs per HEAD (not head-pair): PSUM = scores 2x2 banks
    (double buffered) + av accumulator 2 banks + gemm scratch 2x1 banks
    = 8 banks exactly
  - q/k/v GEMM chunks for pair p+1 and proj passes of the previous batch
    are software-pipelined into the attention mt-loop slots, so the PE
    stays busy while ACT computes exp()
  - softmax 1/denominator via exp(-ln(d)) on ACT (this walrus lacks the
    custom-DVE approx ops; iterative DVE reciprocal costs 6.5us); a DVE
    copy of the numerator releases the av psum accumulator early
  - ones-columns packed next to v in v_ext give the softmax denominator
    for free inside the attention@v matmul (rows 64:128 of the psum)

Explored and rejected (all measured on HW): fp8 (2e-2 tolerance
exceeded: random-sign GEMM error stays ~5.7% relative regardless of N);
PE tile-packing of the K=64 score matmuls (verified ~1.9x overlap on
alternating-row-half pairs via microbenchmark, but a full-array matmul
issued behind a packed pair corrupts the array unless sync-guarded, and
guarded variants measured 641-759us vs 608us -- see kernel_v5.py /
kernel_v4_packed.py); normalize multiply on the Pool engine (Pool
tensor ops ~3x slower than modeled: 742us); deferring the normalize mul
by one head (624us); 1024-col moving matmuls (hardware ISA caps moving
at 512).

This file is self-contained: it monkeypatches two workarounds for the
walrus build in this container (1-sync-wait-per-instruction cap).
"""

import json
import os
import sys
from collections import deque

for _p in ("/opt/trn_rl_repo", os.path.expanduser("~/.axon_site/_ro/trn_rl_repo")):
    if os.path.isdir(_p) and _p not in sys.path:
        sys.path.insert(0, _p)

import numpy as np

import concourse.bass as bass
import concourse.tile as tile
from concourse import mybir
from concourse.bass_utils import run_bass_kernel_spmd
from concourse.vector_clock import ScopedClock
from concourse.masks import make_identity

F32 = mybir.dt.float32
BF16 = mybir.dt.bfloat16
AF = mybir.ActivationFunctionType

# ---------------------------------------------------------------------------
# Workaround: this container's walrus supports at most ONE sync wait per
# instruction.  (a) split the TileContext-exit drain's waits onto single-wait
# NOPs; (b) at BIR-JSON serialization time, hoist extra waits from any
# instruction onto same-engine NOPs placed immediately before it.
# ---------------------------------------------------------------------------

def _patched_drain_and_barrier(self, tick_clock, wait_clock):
    drain_inst = self.nc.sync.drain()
    wait_clock.add_sem_waits(
        drain_inst.ins, ScopedClock({None: tick_clock.global_clock})
    )
    si = drain_inst.ins.sync_info
    waits = list(si.on_wait) if si is not None and si.on_wait else []
    if len(waits) > 1:
        si.on_wait = waits[:1]
        for w in waits[1:]:
            nop = self.nc.sync.nop(hint="drain_wait_split", nofuse=True)
            nsi = nop.ins.sync_info
            if nsi is None:
                nop.ins.sync_info = mybir.SyncInfo(on_wait=[w], on_update=[])
            else:
                nsi.on_wait = list(nsi.on_wait or []) + [w]
    self.nc.all_engine_barrier()
    assert self.sems is not None
    popped = self.nc._tile_sem_poison_stack.pop()
    assert popped is self._sem_poison
    self.nc.clear_and_free_semaphores(list(self.sems.allocated().values()))
    self.nc.all_engine_barrier()


tile.TileContext._drain_and_barrier = _patched_drain_and_barrier


def _split_multi_waits(bir):
    for fn in bir["functions"]:
        for bb in fn["blocks"]:
            new_insts = []
            for inst in bb["instructions"]:
                si = inst.get("sync_info")
                ow = (si or {}).get("on_wait") or []
                if len(ow) > 1:
                    for i, w in enumerate(ow[:-1]):
                        new_insts.append({
                            "debug": inst.get("debug", 0),
                            "engine": inst["engine"],
                            "ins": [], "outs": [],
                            "name": f"{inst['name']}.wsplit{i}",
                            "opcode": "NoOp",
                            "sync_info": {"on_wait": [w], "on_update": []},
                        })
                    si["on_wait"] = [ow[-1]]
                new_insts.append(inst)
            bb["instructions"] = new_insts
    return bir


_orig_to_json_bytes = bass.Bass.to_json_bytes


def _patched_to_json_bytes(self):
    d = json.loads(_orig_to_json_bytes(self))
    _split_multi_waits(d)
    return json.dumps(d).encode()


bass.Bass.to_json_bytes = _patched_to_json_bytes

# ---------------------------------------------------------------------------
# Problem constants (hardcoded per the task contract)
# ---------------------------------------------------------------------------

B, N, C, H, P = 16, 1024, 1024, 16, 16
D = C // H                      # 64
SCALE = float(D) ** -0.5        # 0.125
N_CORES = 8
B_PC = B // N_CORES             # 2 batches per core
NT = N // 128                   # 8 token tiles
CT = C // 128                   # 8 feature tiles
MT = NT + 1                     # 9 m-tiles: tile 0 = prefix (16 valid rows)
HPAIRS = H // 2                 # 8 head pairs
FOLD_NEXT = True


def build_nc(repeat: int = 1) -> bass.Bass:
    nc = bass.Bass()

    x_d = nc.declare_dram_parameter("x", [B_PC, N, C], F32, isOutput=False)
    pk_d = nc.declare_dram_parameter("pk", [B_PC, P, C], F32, isOutput=False)
    pv_d = nc.declare_dram_parameter("pv", [B_PC, P, C], F32, isOutput=False)
    wqkv_d = nc.declare_dram_parameter("w_qkv", [C, 3 * C], F32, isOutput=False)
    wproj_d = nc.declare_dram_parameter("w_proj", [C, C], F32, isOutput=False)
    bias_d = nc.declare_dram_parameter("b_proj", [C], F32, isOutput=False)
    # output is stored TRANSPOSED per batch: [C, N]; host transposes back
    outT_d = nc.declare_dram_parameter("outT", [B_PC, C, N], F32, isOutput=True)

    with tile.TileContext(nc) as tc:
        with tc.tile_pool(name="cons", bufs=1) as cons, \
             tc.tile_pool(name="eP", bufs=3) as e_pool, \
             tc.tile_pool(name="ePre", bufs=2) as epre_pool, \
             tc.tile_pool(name="stg", bufs=1) as stg, \
             tc.tile_pool(name="rbp", bufs=1) as rb_pool, \
             tc.tile_pool(name="xload", bufs=2) as xload, \
             tc.tile_pool(name="xbf", bufs=2) as xbfp, \
             tc.tile_pool(name="osb", bufs=2) as osb, \
             tc.tile_pool(name="psS", bufs=2, space="PSUM") as psS, \
             tc.tile_pool(name="psAV", bufs=1, space="PSUM") as psAV, \
             tc.tile_pool(name="psG", bufs=2, space="PSUM") as psG:

            # ---------------- one-time setup ----------------
            ident_bf = cons.tile([128, 128], BF16, tag="idb")
            make_identity(nc, ident_bf[:])
            # PE warm-up burst: ~3.5us of throwaway matmuls releases the
            # HAM clock-gate (K=4/8 -> 8/8) before the real work arrives,
            # so the preamble transposes/GEMMs run at 2.4 GHz not 1.2.
            warm_ps = psG.tile([128, 128], F32, tag="g", name="warmup")
            for _w in range(32):
                nc.tensor.matmul(
                    warm_ps[:], ident_bf[:], ident_bf[:],
                    start=(_w == 0), stop=(_w == 31),
                )
            # bias in per-partition layout: bias_col[p, cf] = b_proj[cf*128+p]
            bias_col = cons.tile([128, CT], F32, tag="bias")
            nc.sync.dma_start(
                out=bias_col[:],
                in_=bias_d[:].rearrange("(a b) -> b a", b=128),
            )
            # prefix-k staging (bf16 via casting gpsimd DMA)
            pkl = cons.tile([P, C], BF16, tag="pkl")

            # persistent activations (reused across batches; Tile tracks
            # read/write hazards on AP ranges).  qT/kT hold THREE head
            # pairs (slot p%3): pair p+2 is produced by pipelined fillers
            # while pair p's attention reads its slot; the extra slot lets
            # the packed-prefix exp (4 heads = 2 pairs per ACTIVATE) see
            # both of its pairs' q at group start.
            xT = cons.tile([128, CT, N], BF16, tag="xT")
            kT = cons.tile([128, 4, N], BF16, tag="kT")
            qT = cons.tile([128, 4, N], BF16, tag="qT")
            # prefix keys, all pairs: cols 0:16 = pk^T, 16:32 zero so the
            # packed 32-row score stripes come out 0 on rows 16:32 ->
            # exp = 1, harmless because the matching v_ext rows are zero
            kPre = cons.tile([128, HPAIRS, 32], BF16, tag="kPre")
            nc.vector.memset(kPre[:, :, P:32], 0.0)
            oT = cons.tile([128, CT, N], BF16, tag="oT")
            # v_ext[m, mt, h, 0:64] = v values; [.., 64:128] = ones columns
            # (denominator trick). m-tile 0 = prefix, PACKED: head h's 16
            # pv rows live at partitions 32*(h%4)..+16 (matching its stripe
            # in the packed prefix-score psum); all other rows stay ZERO so
            # the other heads' e values in the shared e_pre tile contribute
            # nothing to this head's av or denominator.
            v_ext = cons.tile([128, MT, H, 128], BF16, tag="vx")
            nc.vector.memset(v_ext[:, :, :, 64:128], 1.0)
            nc.vector.memset(v_ext[:, 0, :, :], 0.0)
            for a in range(4):
                nc.vector.memset(
                    v_ext[32 * a:32 * a + P, 0, a::4, 64:128], 1.0
                )

            # weights, bf16, resident for the whole kernel, on the gpsimd
            # sw-DGE queue (the only one that casts).  512-col chunks keep
            # the write packets at 1KB (128-col chunks made 256B packets and
            # left the queue packet-rate-bound for ~60us).  x rides the
            # separate sync HW queue concurrently.
            wq_sb = cons.tile([128, CT, C], BF16, tag="wq")
            wk_sb = cons.tile([128, CT, C], BF16, tag="wk")
            wv_sb = cons.tile([128, CT, C], BF16, tag="wv")
            wp_sb = cons.tile([128, CT, C], BF16, tag="wp")

            def _wload(dst, base, lo, hi):
                nc.gpsimd.dma_start(
                    out=dst[:, :, lo:hi],
                    in_=wqkv_d[:, base + lo:base + hi].rearrange(
                        "(ct p) f -> p ct f", p=128),
                )

            def _pv_load(b):
                pvr = pv_d[b].rearrange("t (h d) -> t h d", d=64)
                for a in range(4):
                    nc.gpsimd.dma_start(
                        out=v_ext[32 * a:32 * a + P, 0, a::4, 0:64],
                        in_=pvr[:, a::4, :],
                    )

            nc.gpsimd.dma_start(out=pkl[:], in_=pk_d[0])
            _wload(wk_sb, C, 0, 128)                  # k pair 0
            _wload(wq_sb, 0, 0, 128)                  # q pair 0
            _wload(wq_sb, 0, 128, 256)                # q pair 1
            _wload(wk_sb, C, 128, 256)                # k pair 1
            _wload(wv_sb, 2 * C, 0, 512)              # v block 0
            _pv_load(0)                               # prefix v, batch 0
            _wload(wv_sb, 2 * C, 512, 1024)           # v block 1
            _wload(wk_sb, C, 256, 640)
            _wload(wq_sb, 0, 256, 640)
            _wload(wk_sb, C, 640, 1024)
            _wload(wq_sb, 0, 640, 1024)
            nc.gpsimd.dma_start(
                out=wp_sb[:],
                in_=wproj_d[:].rearrange("(ct p) f -> p ct f", p=128),
            )

            # ---------------- per-batch work units ----------------

            def qk_units(b, p):
                """4 closures: q and k GEMMs for head pair p, split in two
                512-column halves each. Each accumulates 8 c-tiles into a
                [128,512] psum and copies (cast bf16) into qT/kT."""
                us = []
                for which in ("k", "q"):
                    for jh in range(2):
                        def u(which=which, p=p, jh=jh, b=b):
                            w_sb = wk_sb if which == "k" else wq_sb
                            ps = psG.tile([128, 512], F32, tag="g",
                                          name=f"g{which}_{b}_{p}_{jh}")
                            for ct in range(CT):
                                nc.tensor.matmul(
                                    ps[:],
                                    w_sb[:, ct, p * 128:(p + 1) * 128],
                                    xT[:, ct, jh * 512:(jh + 1) * 512],
                                    start=(ct == 0), stop=(ct == CT - 1),
                                )
                            if which == "k":
                                nc.vector.tensor_copy(
                                    kT[:, p % 4, jh * 512:(jh + 1) * 512],
                                    ps[:],
                                )
                            else:
                                nc.vector.tensor_copy(
                                    qT[:, p % 4, jh * 512:(jh + 1) * 512],
                                    ps[:],
                                )
                        us.append(u)
                return us

            def v_units(b, bk):
                """8 closures: v GEMM for pair block bk (4 pairs = 512 v
                columns), one per token tile. x^T tile is stationary, w_v
                columns are moving -> v lands in NATURAL [token, feature]
                layout, no transpose needed."""
                us = []
                for nt in range(NT):
                    def u(nt=nt, bk=bk, b=b):
                        ps = psG.tile([128, 512], F32, tag="g",
                                      name=f"gv_{b}_{bk}_{nt}")
                        for ct in range(CT):
                            nc.tensor.matmul(
                                ps[:],
                                xT[:, ct, nt * 128:(nt + 1) * 128],
                                wv_sb[:, ct, bk * 512:(bk + 1) * 512],
                                start=(ct == 0), stop=(ct == CT - 1),
                            )
                        nc.vector.tensor_copy(
                            v_ext[:, nt + 1, 8 * bk:8 * (bk + 1), 0:64],
                            ps[:].rearrange("p (h d) -> p h d", d=64),
                        )
                    us.append(u)
                return us

            def proj_units(b):
                """8 closures: one projection f-tile pass each; emitted
                interleaved into the NEXT batch's preamble."""
                us = []
                for cf in range(CT):
                    def u(cf=cf, b=b):
                        ps = psS.tile([128, N], F32, tag="s",
                                      name=f"pp_{b}_{cf}")
                        for ct in range(CT):
                            for j in (0, 512):
                                nc.tensor.matmul(
                                    ps[:, j:j + 512],
                                    wp_sb[:, ct, cf * 128:(cf + 1) * 128],
                                    oT[:, ct, j:j + 512],
                                    start=(ct == 0), stop=(ct == CT - 1),
                                )
                        o_sb = osb.tile([128, N], F32, tag="o",
                                        name=f"osb_{b}_{cf}")
                        nc.vector.tensor_scalar_add(
                            o_sb[:], ps[:], bias_col[:, cf:cf + 1]
                        )
                        nc.sync.dma_start(
                            out=outT_d[b, cf * 128:(cf + 1) * 128, :],
                            in_=o_sb[:],
                        )
                    us.append(u)
                return us

            def tile_unit(b, nt):
                """x tile -> bf16 -> x^T (sync-HW-queue DMA, ACT cast, PE
                transposes).  Returned as a closure so the NEXT batch's
                tiles can run as lazy filler in THIS batch's tail pairs.
                (XBAR DMA transpose was measured: 208B packets, 855us total
                -- the PE path is far faster for 128x128 tiles.)"""
                def u():
                    xl = xload.tile([128, C], F32, tag="xl",
                                    name=f"xl_{b}_{nt}")
                    nc.sync.dma_start(
                        out=xl[:], in_=x_d[b, nt * 128:(nt + 1) * 128, :]
                    )
                    xbf = xbfp.tile([128, C], BF16, tag="xbf",
                                    name=f"xbf_{b}_{nt}")
                    nc.scalar.activation(xbf[:], xl[:], AF.Copy)
                    ps_t = psG.tile([128, CT, 128], BF16, tag="g",
                                    name=f"pst_{b}_{nt}")
                    for ct in range(CT):
                        nc.tensor.transpose(
                            ps_t[:, ct, :],
                            xbf[:, ct * 128:(ct + 1) * 128],
                            ident_bf[:],
                        )
                    nc.vector.tensor_copy(
                        xT[:, :, nt * 128:(nt + 1) * 128], ps_t[:]
                    )
                return u

            def emit_batch(b, carry):
                """Emit one batch; `carry` = proj closures of the previous
                batch, interleaved into this batch's preamble. Returns this
                batch's proj closures."""
                units = deque(carry)

                def drain(k=1):
                    for _ in range(k):
                        if units:
                            units.popleft()()

                if b > 0:
                    # prefix staging for this batch (casting gpsimd DMAs;
                    # the gpsimd engine reaches these while the previous
                    # attention still runs -> prefetch)
                    nc.gpsimd.dma_start(out=pkl[:], in_=pk_d[b])

                vb0 = v_units(b, 0)
                # the qk GEMM for token half jh only reads xT columns
                # jh*512..+512 (= x tiles 4jh..4jh+3), so half the qk
                # and v work starts after only FOUR tiles are
                # transposed -- the PE chews on it while tiles 4-7
                # stream in.
                qk0 = qk_units(b, 0)   # [k-jh0, k-jh1, q-jh0, q-jh1]
                qk1 = qk_units(b, 1)
                for nt in range(4):
                    tile_unit(b, nt)()
                    drain(1)
                for u in (qk0[0], qk0[2], qk1[0], qk1[2]):
                    u()
                    drain(1)
                for nt in range(4):
                    vb0[nt]()
                    if nt < 2:
                        tile_unit(b, 4 + nt)()
                    drain(1)
                tile_unit(b, 6)()
                tile_unit(b, 7)()
                # prefix: pk^T into kPre cols 0:16
                ps_pk = psG.tile([128, CT, P], BF16, tag="g",
                                 name=f"pspk_{b}")
                for ct in range(CT):
                    nc.tensor.transpose(
                        ps_pk[:, ct, :],
                        pkl[:, ct * 128:(ct + 1) * 128],
                        ident_bf[0:P, 0:P],
                    )
                nc.vector.tensor_copy(kPre[:, :, 0:P], ps_pk[:])
                if b > 0:
                    _pv_load(b)
                for u in (qk0[1], qk0[3], qk1[1], qk1[3]):
                    u()
                    drain(1)
                for nt in range(4, NT):
                    vb0[nt]()
                    drain(1)
                drain(len(units))  # force out any remaining carry

                def prefix_group(g):
                    """Packed prefix scores for heads 4g..4g+3 (pairs 2g,
                    2g+1): head h's 16 prefix keys land on psum rows
                    32*(h%4)..+32 (stationary is 32 wide, cols 16:32 zero),
                    so ONE exp serves 4 heads.  MMs are ordered row-half-
                    major so only verified-safe masked||masked overlap can
                    occur."""
                    ps_pre = psS.tile([128, N], F32, tag="s",
                                      name=f"pre_{b}_{g}")
                    for hh in (0, 1):
                        base = hh * 64
                        for hg in (hh, hh + 2):
                            h = 4 * g + hg
                            p = h // 2
                            for j in (0, 512):
                                nc.tensor.matmul(
                                    ps_pre[32 * hg:32 * hg + 32, j:j + 512],
                                    kPre[base:base + D, p, :],
                                    qT[base:base + D, p % 4, j:j + 512],
                                    start=True, stop=True,
                                    tile_position=(base, 32 * hg),
                                )
                    e_pre = epre_pool.tile([128, N], BF16, tag="ep",
                                           name=f"ep_{b}_{g}")
                    nc.scalar.activation(e_pre[:], ps_pre[:], AF.Exp,
                                         scale=SCALE)
                    return e_pre

                e_pre = prefix_group(0)

                # ---- per-head attention, gemm pipeline in the slots.
                # urgent = next-next pair's q/k (deadline: pair p+1 end);
                # lazy = v block 1 (deadline: pair 4) ----
                urgent = deque()
                lazy = deque()
                for p in range(HPAIRS):
                    if p + 2 < HPAIRS:
                        urgent.extend(qk_units(b, p + 2))
                    if p == 0:
                        lazy.extend(v_units(b, 1))
                    if p >= 2 and p % 2 == 0:
                        e_pre = prefix_group(p // 2)
                    lazy_budget = 2
                    slot = 0
                    for hh in range(2):
                        base = hh * 64
                        h = 2 * p + hh
                        ps_av = psAV.tile([128, N], F32, tag="av",
                                          name=f"av_{b}_{h}")
                        # prefix contribution from the shared packed exp
                        for j in (0, 512):
                            nc.tensor.matmul(
                                ps_av[:, j:j + 512],
                                v_ext[:, 0, h, :],
                                e_pre[:, j:j + 512],
                                start=True, stop=False,
                            )
                        for mt in range(1, MT):
                            ps_s = psS.tile([128, N], F32, tag="s",
                                            name=f"s_{b}_{h}_{mt}")
                            for j in (0, 512):
                                nc.tensor.matmul(
                                    ps_s[:, j:j + 512],
                                    kT[base:base + D, p % 4,
                                       (mt - 1) * 128:mt * 128],
                                    qT[base:base + D, p % 4, j:j + 512],
                                    start=True, stop=True,
                                )
                            eT = e_pool.tile([128, N], BF16, tag="e",
                                             name=f"e_{b}_{h}_{mt}")
                            nc.scalar.activation(eT[:], ps_s[:], AF.Exp,
                                                 scale=SCALE)
                            # gemm/proj filler BETWEEN exp and av: the PE
                            # would otherwise idle waiting for the exp (and,
                            # at mt==1, for the previous head's psum release)
                            slot += 1
                            if urgent and (mt in (1, 5)
                                           or len(urgent) >= 18 - slot):
                                urgent.popleft()()
                            elif lazy and lazy_budget > 0 and mt in (3, 7):
                                lazy.popleft()()
                                lazy_budget -= 1
                            for j in (0, 512):
                                nc.tensor.matmul(
                                    ps_av[:, j:j + 512],
                                    v_ext[:, mt, h, :],
                                    eT[:, j:j + 512],
                                    start=False, stop=(mt == MT - 1),
                                )
                        # normalize: out = unnorm * exp(-ln(denom)).
                        # (custom-DVE reciprocal_approx is unsupported by this
                        # walrus; iterative DVE reciprocal costs 6.5us.)
                        # The numerator is copied to SBUF so the psum
                        # accumulator is released after ~1.1us (copy || ln)
                        # instead of after the full ln->exp->mul chain.
                        num_sb = stg.tile([64, N], F32, tag="st",
                                          name=f"st_{b}_{h}")
                        nc.vector.tensor_copy(num_sb[:], ps_av[0:64, :])
                        lnd = rb_pool.tile([64, N], F32, tag="ln",
                                           name=f"ln_{b}_{h}")
                        nc.scalar.activation(lnd[:], ps_av[64:128, :], AF.Ln)
                        rb = rb_pool.tile([64, N], F32, tag="rb",
                                          name=f"rb_{b}_{h}")
                        nc.scalar.activation(rb[:], lnd[:], AF.Exp,
                                             scale=-1.0)
                        nc.vector.tensor_mul(
                            oT[base:base + D, p, :], num_sb[:], rb[:]
                        )
                    if p >= HPAIRS - 3:
                        # tail: no further slots are guaranteed, flush
                        while urgent:
                            urgent.popleft()()
                        while lazy:
                            lazy.popleft()()

                return proj_units(b)

            carry = []
            for _rep in range(repeat):
                for b in range(B_PC):
                    carry = emit_batch(b, carry)
            for u in carry:
                u()

    return nc


_NC_CACHE = {}


def _get_nc(repeat: int = 1) -> bass.Bass:
    key = f"nc{repeat}"
    if key not in _NC_CACHE:
        _NC_CACHE[key] = build_nc(repeat)
    return _NC_CACHE[key]


def _make_runner(nc):
    """Compile the SPMD kernel ONCE into a reusable callable.

    Mirrors bass2jax.run_bass_via_pjrt's multi-core branch, but without
    output-buffer donation so the compiled function + device-resident
    inputs can be invoked repeatedly (for wall-clock benchmarking and to
    avoid recompiles on every kernel() call).
    """
    import jax
    from jax.experimental.shard_map import shard_map
    from jax.sharding import Mesh, PartitionSpec
    from concourse import bass2jax
    from concourse.bass2jax import _bass_exec_p, partition_id_tensor

    bass2jax.install_neuronx_cc_hook()

    partition_name = (
        nc.partition_id_tensor.name if nc.partition_id_tensor else None
    )
    in_names, out_names, out_avals, zero_outs = [], [], [], []
    for alloc in nc.m.functions[0].allocations:
        if not isinstance(alloc, mybir.MemoryLocationSet):
            continue
        name = alloc.memorylocations[0].name
        if alloc.kind == "ExternalInput":
            if name != partition_name:
                in_names.append(name)
        elif alloc.kind == "ExternalOutput":
            shape = tuple(alloc.tensor_shape)
            dtype = mybir.dt.np(alloc.dtype)
            out_names.append(name)
            out_avals.append(jax.core.ShapedArray(shape, dtype))
            zero_outs.append(np.zeros(shape, dtype))
    n_params = len(in_names)
    all_in_names = list(in_names) + list(out_names)
    if partition_name is not None:
        all_in_names.append(partition_name)

    def _body(*args):
        operands = list(args)
        if partition_name is not None:
            operands.append(partition_id_tensor())
        outs = _bass_exec_p.bind(
            *operands,
            out_avals=tuple(out_avals),
            in_names=tuple(all_in_names),
            out_names=tuple(out_names),
            lowering_input_output_aliases=(),
            sim_require_finite=True,
            sim_require_nnan=True,
            nc=nc,
        )
        return tuple(outs)

    devices = jax.devices()[:N_CORES]
    mesh = Mesh(np.asarray(devices), ("core",))
    n_outs = len(out_avals)
    in_specs = (PartitionSpec("core"),) * (n_params + n_outs)
    out_specs = (PartitionSpec("core"),) * n_outs
    sharded = jax.jit(
        shard_map(_body, mesh=mesh, in_specs=in_specs,
                  out_specs=out_specs, check_rep=False),
        keep_unused=True,
    )

    concat_zeros = [
        np.zeros((N_CORES * z.shape[0], *z.shape[1:]), z.dtype)
        for z in zero_outs
    ]

    state = {"dev_zeros": None}

    def runner(in_maps):
        per_core = [
            [np.asarray(m[name]) for name in in_names] for m in in_maps
        ]
        concat_in = [
            np.concatenate([per_core[c][i] for c in range(N_CORES)], axis=0)
            for i in range(n_params)
        ]
        if state["dev_zeros"] is None:
            state["dev_zeros"] = [jax.device_put(z) for z in concat_zeros]
        out_arrs = sharded(*concat_in, *state["dev_zeros"])
        return [
            {
                name: np.asarray(out_arrs[i]).reshape(
                    N_CORES, *out_avals[i].shape
                )[c]
                for i, name in enumerate(out_names)
            }
            for c in range(N_CORES)
        ]

    def runner_dev(dev_args):
        """dev_args: device-resident concat inputs; returns device outputs."""
        return sharded(*dev_args, *state["dev_zeros"])

    def make_dev_args(in_maps):
        per_core = [
            [np.asarray(m[name]) for name in in_names] for m in in_maps
        ]
        concat_in = [
            np.concatenate([per_core[c][i] for c in range(N_CORES)], axis=0)
            for i in range(n_params)
        ]
        if state["dev_zeros"] is None:
            state["dev_zeros"] = [jax.device_put(z) for z in concat_zeros]
        return [jax.device_put(a) for a in concat_in]

    return runner, runner_dev, make_dev_args


def _get_runner(repeat: int = 1):
    key = f"runner{repeat}"
    if key not in _NC_CACHE:
        _NC_CACHE[key] = _make_runner(_get_nc(repeat))
    return _NC_CACHE[key]


def _make_in_maps(x, pk, pv, w_qkv, w_proj, b_proj):
    x = np.ascontiguousarray(np.asarray(x, dtype=np.float32))
    pk = np.ascontiguousarray(np.asarray(pk, dtype=np.float32))
    pv = np.ascontiguousarray(np.asarray(pv, dtype=np.float32))
    w_qkv = np.ascontiguousarray(np.asarray(w_qkv, dtype=np.float32))
    w_proj = np.ascontiguousarray(np.asarray(w_proj, dtype=np.float32))
    b_proj = np.ascontiguousarray(np.asarray(b_proj, dtype=np.float32))
    in_maps = []
    for c in range(N_CORES):
        sl = slice(c * B_PC, (c + 1) * B_PC)
        in_maps.append({
            "x": x[sl], "pk": pk[sl], "pv": pv[sl],
            "w_qkv": w_qkv, "w_proj": w_proj, "b_proj": b_proj,
        })
    return in_maps


def run(x, pk, pv, w_qkv, w_proj, b_proj, trace=False, **trace_kwargs):
    """Run the SPMD kernel; returns (output [B,N,C], results).

    With trace=True, routes through run_bass_kernel_spmd so the returned
    results object carries .exec_time_ns / .profile_json.
    """
    in_maps = _make_in_maps(x, pk, pv, w_qkv, w_proj, b_proj)
    if trace:
        res = run_bass_kernel_spmd(
            _get_nc(), in_maps, list(range(N_CORES)), trace=True,
            **trace_kwargs,
        )
        results = res.results
        out = np.empty((B, N, C), dtype=np.float32)
        for c in range(N_CORES):
            outT = results[c]["outT"]          # [B_PC, C, N]
            out[c * B_PC:(c + 1) * B_PC] = outT.transpose(0, 2, 1)
        return out, res
    runner, _, _ = _get_runner()
    results = runner(in_maps)
    out = np.empty((B, N, C), dtype=np.float32)
    for c in range(N_CORES):
        outT = results[c]["outT"]              # [B_PC, C, N]
        out[c * B_PC:(c + 1) * B_PC] = outT.transpose(0, 2, 1)
    return out, results


def kernel(x, pk, pv, w_qkv, w_proj, b_proj) -> np.ndarray:
    out, _ = run(x, pk, pv, w_qkv, w_proj, b_proj)
    return out


def benchmark(x, pk, pv, w_qkv, w_proj, b_proj, iters=20, warmup=3, repeat=1):
    """Median wall-clock per executed call with device-resident inputs."""
    import time
    import jax
    _, runner_dev, make_dev_args = _get_runner(repeat)
    in_maps = _make_in_maps(x, pk, pv, w_qkv, w_proj, b_proj)
    dev_args = make_dev_args(in_maps)
    for _ in range(warmup):
        outs = runner_dev(dev_args)
        jax.block_until_ready(outs)
    ts = []
    for _ in range(iters):
        t0 = time.perf_counter()
        outs = runner_dev(dev_args)
        jax.block_until_ready(outs)
        ts.append(time.perf_counter() - t0)
    ts.sort()
    return {
        "median_s": ts[len(ts) // 2],
        "min_s": ts[0],
        "all_s": ts,
    }



# revision 35
# speedup vs baseline: 1.0391x; 1.0016x over previous
"""Trainium2 Bass kernel for prefix-KV multi-head attention (v2).

Reference computation (per batch):
    qkv = x @ w_qkv -> q,k,v heads; k/v get a 16-token prefix (pk, pv)
    attn = softmax(q @ k^T * D^-0.5); out = (attn @ v) @ w_proj + b_proj

Sharding: data-parallel over B across 8 NeuronCores (2 batches per core).

Design (vs the v1 baseline, 700us -> 608us):
  - weights loaded to SBUF once per core (bf16), reused by both batches
  - q^T kept in SBUF (no DRAM spill)
  - v computed in NATURAL [token, feature] layout via x^T-stationary GEMM
    (moving = w_v columns), eliminating all per-head v transposes
  - x^T built with bf16 PE transposes (2x faster than fp32)
  - attention runs per HEAD (not head-pair): PSUM = scores 2x2 banks
    (double buffered) + av accumulator 2 banks + gemm scratch 2x1 banks
    = 8 banks exactly
  - q/k/v GEMM chunks for pair p+1 and proj passes of the previous batch
    are software-pipelined into the attention mt-loop slots, so the PE
    stays busy while ACT computes exp()
  - softmax 1/denominator via exp(-ln(d)) on ACT (this walrus lacks the
    custom-DVE approx ops; iterative DVE reciprocal costs 6.5us); a DVE
    copy of the numerator releases the av psum accumulator early
  - ones-columns packed next to v in v_ext give the softmax denominator
    for free inside the attention@v matmul (rows 64:128 of the psum)

Explored and rejected (all measured on HW): fp8 (2e-2 tolerance
exceeded: random-sign GEMM error stays ~5.7% relative regardless of N);
PE tile-packing of the K=64 score matmuls (verified ~1.9x overlap on
alternating-row-half pairs via microbenchmark, but a full-array matmul
issued behind a packed pair corrupts the array unless sync-guarded, and
guarded variants measured 641-759us vs 608us -- see kernel_v5.py /
kernel_v4_packed.py); normalize multiply on the Pool engine (Pool
tensor ops ~3x slower than modeled: 742us); deferring the normalize mul
by one head (624us); 1024-col moving matmuls (hardware ISA caps moving
at 512).

This file is self-contained: it monkeypatches two workarounds for the
walrus build in this container (1-sync-wait-per-instruction cap).
"""

import json
import os
import sys
from collections import deque

for _p in ("/opt/trn_rl_repo", os.path.expanduser("~/.axon_site/_ro/trn_rl_repo")):
    if os.path.isdir(_p) and _p not in sys.path:
        sys.path.insert(0, _p)

import numpy as np

import concourse.bass as bass
import concourse.tile as tile
from concourse import mybir
from concourse.bass_utils import run_bass_kernel_spmd
from concourse.vector_clock import ScopedClock
from concourse.masks import make_identity

F32 = mybir.dt.float32
BF16 = mybir.dt.bfloat16
AF = mybir.ActivationFunctionType

# ---------------------------------------------------------------------------
# Workaround: this container's walrus supports at most ONE sync wait per
# instruction.  (a) split the TileContext-exit drain's waits onto single-wait
# NOPs; (b) at BIR-JSON serialization time, hoist extra waits from any
# instruction onto same-engine NOPs placed immediately before it.
# ---------------------------------------------------------------------------

def _patched_drain_and_barrier(self, tick_clock, wait_clock):
    drain_inst = self.nc.sync.drain()
    wait_clock.add_sem_waits(
        drain_inst.ins, ScopedClock({None: tick_clock.global_clock})
    )
    si = drain_inst.ins.sync_info
    waits = list(si.on_wait) if si is not None and si.on_wait else []
    if len(waits) > 1:
        si.on_wait = waits[:1]
        for w in waits[1:]:
            nop = self.nc.sync.nop(hint="drain_wait_split", nofuse=True)
            nsi = nop.ins.sync_info
            if nsi is None:
                nop.ins.sync_info = mybir.SyncInfo(on_wait=[w], on_update=[])
            else:
                nsi.on_wait = list(nsi.on_wait or []) + [w]
    self.nc.all_engine_barrier()
    assert self.sems is not None
    popped = self.nc._tile_sem_poison_stack.pop()
    assert popped is self._sem_poison
    self.nc.clear_and_free_semaphores(list(self.sems.allocated().values()))
    self.nc.all_engine_barrier()


tile.TileContext._drain_and_barrier = _patched_drain_and_barrier


def _split_multi_waits(bir):
    for fn in bir["functions"]:
        for bb in fn["blocks"]:
            new_insts = []
            for inst in bb["instructions"]:
                si = inst.get("sync_info")
                ow = (si or {}).get("on_wait") or []
                if len(ow) > 1:
                    for i, w in enumerate(ow[:-1]):
                        new_insts.append({
                            "debug": inst.get("debug", 0),
                            "engine": inst["engine"],
                            "ins": [], "outs": [],
                            "name": f"{inst['name']}.wsplit{i}",
                            "opcode": "NoOp",
                            "sync_info": {"on_wait": [w], "on_update": []},
                        })
                    si["on_wait"] = [ow[-1]]
                new_insts.append(inst)
            bb["instructions"] = new_insts
    return bir


_orig_to_json_bytes = bass.Bass.to_json_bytes


def _patched_to_json_bytes(self):
    d = json.loads(_orig_to_json_bytes(self))
    _split_multi_waits(d)
    return json.dumps(d).encode()


bass.Bass.to_json_bytes = _patched_to_json_bytes

# ---------------------------------------------------------------------------
# Problem constants (hardcoded per the task contract)
# ---------------------------------------------------------------------------

B, N, C, H, P = 16, 1024, 1024, 16, 16
D = C // H                      # 64
SCALE = float(D) ** -0.5        # 0.125
N_CORES = 8
B_PC = B // N_CORES             # 2 batches per core
NT = N // 128                   # 8 token tiles
CT = C // 128                   # 8 feature tiles
MT = NT + 1                     # 9 m-tiles: tile 0 = prefix (16 valid rows)
HPAIRS = H // 2                 # 8 head pairs
FOLD_NEXT = True


def build_nc(repeat: int = 1) -> bass.Bass:
    nc = bass.Bass()

    x_d = nc.declare_dram_parameter("x", [B_PC, N, C], F32, isOutput=False)
    pk_d = nc.declare_dram_parameter("pk", [B_PC, P, C], F32, isOutput=False)
    pv_d = nc.declare_dram_parameter("pv", [B_PC, P, C], F32, isOutput=False)
    wqkv_d = nc.declare_dram_parameter("w_qkv", [C, 3 * C], F32, isOutput=False)
    wproj_d = nc.declare_dram_parameter("w_proj", [C, C], F32, isOutput=False)
    bias_d = nc.declare_dram_parameter("b_proj", [C], F32, isOutput=False)
    # output is stored TRANSPOSED per batch: [C, N]; host transposes back
    outT_d = nc.declare_dram_parameter("outT", [B_PC, C, N], F32, isOutput=True)

    with tile.TileContext(nc) as tc:
        with tc.tile_pool(name="cons", bufs=1) as cons, \
             tc.tile_pool(name="eP", bufs=4) as e_pool, \
             tc.tile_pool(name="ePre", bufs=2) as epre_pool, \
             tc.tile_pool(name="stg", bufs=1) as stg, \
             tc.tile_pool(name="rbp", bufs=1) as rb_pool, \
             tc.tile_pool(name="xload", bufs=3) as xload, \
             tc.tile_pool(name="xbf", bufs=2) as xbfp, \
             tc.tile_pool(name="osb", bufs=2) as osb, \
             tc.tile_pool(name="psS", bufs=2, space="PSUM") as psS, \
             tc.tile_pool(name="psAV", bufs=1, space="PSUM") as psAV, \
             tc.tile_pool(name="psG", bufs=2, space="PSUM") as psG:

            # ---------------- one-time setup ----------------
            ident_bf = cons.tile([128, 128], BF16, tag="idb")
            make_identity(nc, ident_bf[:])
            # PE warm-up burst: ~3.5us of throwaway matmuls releases the
            # HAM clock-gate (K=4/8 -> 8/8) before the real work arrives,
            # so the preamble transposes/GEMMs run at 2.4 GHz not 1.2.
            warm_ps = psG.tile([128, 128], F32, tag="g", name="warmup")
            for _w in range(32):
                nc.tensor.matmul(
                    warm_ps[:], ident_bf[:], ident_bf[:],
                    start=(_w == 0), stop=(_w == 31),
                )
            # bias in per-partition layout: bias_col[p, cf] = b_proj[cf*128+p]
            bias_col = cons.tile([128, CT], F32, tag="bias")
            nc.sync.dma_start(
                out=bias_col[:],
                in_=bias_d[:].rearrange("(a b) -> b a", b=128),
            )
            # prefix-k staging (bf16 via casting gpsimd DMA)
            pkl = cons.tile([P, C], BF16, tag="pkl")

            # persistent activations (reused across batches; Tile tracks
            # read/write hazards on AP ranges).  qT/kT hold THREE head
            # pairs (slot p%3): pair p+2 is produced by pipelined fillers
            # while pair p's attention reads its slot; the extra slot lets
            # the packed-prefix exp (4 heads = 2 pairs per ACTIVATE) see
            # both of its pairs' q at group start.
            xT = cons.tile([128, CT, N], BF16, tag="xT")
            kT = cons.tile([128, 4, N], BF16, tag="kT")
            qT = cons.tile([128, 4, N], BF16, tag="qT")
            # prefix keys, all pairs: cols 0:16 = pk^T, 16:32 zero so the
            # packed 32-row score stripes come out 0 on rows 16:32 ->
            # exp = 1, harmless because the matching v_ext rows are zero
            kPre = cons.tile([128, HPAIRS, 32], BF16, tag="kPre")
            nc.vector.memset(kPre[:, :, P:32], 0.0)
            oT = cons.tile([128, CT, N], BF16, tag="oT")
            # v_ext[m, mt, h, 0:64] = v values; [.., 64:128] = ones columns
            # (denominator trick). m-tile 0 = prefix, PACKED: head h's 16
            # pv rows live at partitions 32*(h%4)..+16 (matching its stripe
            # in the packed prefix-score psum); all other rows stay ZERO so
            # the other heads' e values in the shared e_pre tile contribute
            # nothing to this head's av or denominator.
            v_ext = cons.tile([128, MT, H, 128], BF16, tag="vx")
            nc.vector.memset(v_ext[:, :, :, 64:128], 1.0)
            nc.vector.memset(v_ext[:, 0, :, :], 0.0)
            for a in range(4):
                nc.vector.memset(
                    v_ext[32 * a:32 * a + P, 0, a::4, 64:128], 1.0
                )

            # weights, bf16, resident for the whole kernel, on the gpsimd
            # sw-DGE queue (the only one that casts).  512-col chunks keep
            # the write packets at 1KB (128-col chunks made 256B packets and
            # left the queue packet-rate-bound for ~60us).  x rides the
            # separate sync HW queue concurrently.
            wq_sb = cons.tile([128, CT, C], BF16, tag="wq")
            wk_sb = cons.tile([128, CT, C], BF16, tag="wk")
            wv_sb = cons.tile([128, CT, C], BF16, tag="wv")
            wp_sb = cons.tile([128, CT, C], BF16, tag="wp")

            def _wload(dst, base, lo, hi):
                nc.gpsimd.dma_start(
                    out=dst[:, :, lo:hi],
                    in_=wqkv_d[:, base + lo:base + hi].rearrange(
                        "(ct p) f -> p ct f", p=128),
                )

            def _pv_load(b):
                pvr = pv_d[b].rearrange("t (h d) -> t h d", d=64)
                for a in range(4):
                    nc.gpsimd.dma_start(
                        out=v_ext[32 * a:32 * a + P, 0, a::4, 0:64],
                        in_=pvr[:, a::4, :],
                    )

            nc.gpsimd.dma_start(out=pkl[:], in_=pk_d[0])
            _wload(wk_sb, C, 0, 128)                  # k pair 0
            _wload(wq_sb, 0, 0, 128)                  # q pair 0
            _wload(wq_sb, 0, 128, 256)                # q pair 1
            _wload(wk_sb, C, 128, 256)                # k pair 1
            _wload(wv_sb, 2 * C, 0, 512)              # v block 0
            _pv_load(0)                               # prefix v, batch 0
            _wload(wv_sb, 2 * C, 512, 1024)           # v block 1
            _wload(wk_sb, C, 256, 640)
            _wload(wq_sb, 0, 256, 640)
            _wload(wk_sb, C, 640, 1024)
            _wload(wq_sb, 0, 640, 1024)
            nc.gpsimd.dma_start(
                out=wp_sb[:],
                in_=wproj_d[:].rearrange("(ct p) f -> p ct f", p=128),
            )

            # ---------------- per-batch work units ----------------

            def qk_units(b, p):
                """4 closures: q and k GEMMs for head pair p, split in two
                512-column halves each. Each accumulates 8 c-tiles into a
                [128,512] psum and copies (cast bf16) into qT/kT."""
                us = []
                for which in ("k", "q"):
                    for jh in range(2):
                        def u(which=which, p=p, jh=jh, b=b):
                            w_sb = wk_sb if which == "k" else wq_sb
                            ps = psG.tile([128, 512], F32, tag="g",
                                          name=f"g{which}_{b}_{p}_{jh}")
                            for ct in range(CT):
                                nc.tensor.matmul(
                                    ps[:],
                                    w_sb[:, ct, p * 128:(p + 1) * 128],
                                    xT[:, ct, jh * 512:(jh + 1) * 512],
                                    start=(ct == 0), stop=(ct == CT - 1),
                                )
                            if which == "k":
                                nc.vector.tensor_copy(
                                    kT[:, p % 4, jh * 512:(jh + 1) * 512],
                                    ps[:],
                                )
                            else:
                                nc.vector.tensor_copy(
                                    qT[:, p % 4, jh * 512:(jh + 1) * 512],
                                    ps[:],
                                )
                        us.append(u)
                return us

            def v_units(b, bk):
                """8 closures: v GEMM for pair block bk (4 pairs = 512 v
                columns), one per token tile. x^T tile is stationary, w_v
                columns are moving -> v lands in NATURAL [token, feature]
                layout, no transpose needed."""
                us = []
                for nt in range(NT):
                    def u(nt=nt, bk=bk, b=b):
                        ps = psG.tile([128, 512], F32, tag="g",
                                      name=f"gv_{b}_{bk}_{nt}")
                        for ct in range(CT):
                            nc.tensor.matmul(
                                ps[:],
                                xT[:, ct, nt * 128:(nt + 1) * 128],
                                wv_sb[:, ct, bk * 512:(bk + 1) * 512],
                                start=(ct == 0), stop=(ct == CT - 1),
                            )
                        nc.vector.tensor_copy(
                            v_ext[:, nt + 1, 8 * bk:8 * (bk + 1), 0:64],
                            ps[:].rearrange("p (h d) -> p h d", d=64),
                        )
                    us.append(u)
                return us

            def proj_units(b):
                """8 closures: one projection f-tile pass each; emitted
                interleaved into the NEXT batch's preamble."""
                us = []
                for cf in range(CT):
                    def u(cf=cf, b=b):
                        ps = psS.tile([128, N], F32, tag="s",
                                      name=f"pp_{b}_{cf}")
                        for ct in range(CT):
                            for j in (0, 512):
                                nc.tensor.matmul(
                                    ps[:, j:j + 512],
                                    wp_sb[:, ct, cf * 128:(cf + 1) * 128],
                                    oT[:, ct, j:j + 512],
                                    start=(ct == 0), stop=(ct == CT - 1),
                                )
                        o_sb = osb.tile([128, N], F32, tag="o",
                                        name=f"osb_{b}_{cf}")
                        nc.vector.tensor_scalar_add(
                            o_sb[:], ps[:], bias_col[:, cf:cf + 1]
                        )
                        nc.sync.dma_start(
                            out=outT_d[b, cf * 128:(cf + 1) * 128, :],
                            in_=o_sb[:],
                        )
                    us.append(u)
                return us

            def tile_unit(b, nt):
                """x tile -> bf16 -> x^T (sync-HW-queue DMA, ACT cast, PE
                transposes).  Returned as a closure so the NEXT batch's
                tiles can run as lazy filler in THIS batch's tail pairs.
                (XBAR DMA transpose was measured: 208B packets, 855us total
                -- the PE path is far faster for 128x128 tiles.)"""
                def u():
                    xl = xload.tile([128, C], F32, tag="xl",
                                    name=f"xl_{b}_{nt}")
                    nc.sync.dma_start(
                        out=xl[:], in_=x_d[b, nt * 128:(nt + 1) * 128, :]
                    )
                    xbf = xbfp.tile([128, C], BF16, tag="xbf",
                                    name=f"xbf_{b}_{nt}")
                    nc.scalar.activation(xbf[:], xl[:], AF.Copy)
                    ps_t = psG.tile([128, CT, 128], BF16, tag="g",
                                    name=f"pst_{b}_{nt}")
                    for ct in range(CT):
                        nc.tensor.transpose(
                            ps_t[:, ct, :],
                            xbf[:, ct * 128:(ct + 1) * 128],
                            ident_bf[:],
                        )
                    nc.vector.tensor_copy(
                        xT[:, :, nt * 128:(nt + 1) * 128], ps_t[:]
                    )
                return u

            def emit_batch(b, carry):
                """Emit one batch; `carry` = proj closures of the previous
                batch, interleaved into this batch's preamble. Returns this
                batch's proj closures."""
                units = deque(carry)

                def drain(k=1):
                    for _ in range(k):
                        if units:
                            units.popleft()()

                if b > 0:
                    # prefix staging for this batch (casting gpsimd DMAs;
                    # the gpsimd engine reaches these while the previous
                    # attention still runs -> prefetch)
                    nc.gpsimd.dma_start(out=pkl[:], in_=pk_d[b])

                vb0 = v_units(b, 0)
                # the qk GEMM for token half jh only reads xT columns
                # jh*512..+512 (= x tiles 4jh..4jh+3), so half the qk
                # and v work starts after only FOUR tiles are
                # transposed -- the PE chews on it while tiles 4-7
                # stream in.
                qk0 = qk_units(b, 0)   # [k-jh0, k-jh1, q-jh0, q-jh1]
                qk1 = qk_units(b, 1)
                for nt in range(4):
                    tile_unit(b, nt)()
                    drain(1)
                for u in (qk0[0], qk0[2], qk1[0], qk1[2]):
                    u()
                    drain(1)
                for nt in range(4):
                    vb0[nt]()
                    if nt < 2:
                        tile_unit(b, 4 + nt)()
                    drain(1)
                tile_unit(b, 6)()
                tile_unit(b, 7)()
                # prefix: pk^T into kPre cols 0:16
                ps_pk = psG.tile([128, CT, P], BF16, tag="g",
                                 name=f"pspk_{b}")
                for ct in range(CT):
                    nc.tensor.transpose(
                        ps_pk[:, ct, :],
                        pkl[:, ct * 128:(ct + 1) * 128],
                        ident_bf[0:P, 0:P],
                    )
                nc.vector.tensor_copy(kPre[:, :, 0:P], ps_pk[:])
                if b > 0:
                    _pv_load(b)
                for u in (qk0[1], qk0[3], qk1[1], qk1[3]):
                    u()
                    drain(1)
                for nt in range(4, NT):
                    vb0[nt]()
                    drain(1)
                drain(len(units))  # force out any remaining carry

                def prefix_group(g):
                    """Packed prefix scores for heads 4g..4g+3 (pairs 2g,
                    2g+1): head h's 16 prefix keys land on psum rows
                    32*(h%4)..+32 (stationary is 32 wide, cols 16:32 zero),
                    so ONE exp serves 4 heads.  MMs are ordered row-half-
                    major so only verified-safe masked||masked overlap can
                    occur."""
                    ps_pre = psS.tile([128, N], F32, tag="s",
                                      name=f"pre_{b}_{g}")
                    for hh in (0, 1):
                        base = hh * 64
                        for hg in (hh, hh + 2):
                            h = 4 * g + hg
                            p = h // 2
                            for j in (0, 512):
                                nc.tensor.matmul(
                                    ps_pre[32 * hg:32 * hg + 32, j:j + 512],
                                    kPre[base:base + D, p, :],
                                    qT[base:base + D, p % 4, j:j + 512],
                                    start=True, stop=True,
                                    tile_position=(base, 32 * hg),
                                )
                    e_pre = epre_pool.tile([128, N], BF16, tag="ep",
                                           name=f"ep_{b}_{g}")
                    nc.scalar.activation(e_pre[:], ps_pre[:], AF.Exp,
                                         scale=SCALE)
                    return e_pre

                e_pre = prefix_group(0)

                # ---- per-head attention, gemm pipeline in the slots.
                # urgent = next-next pair's q/k (deadline: pair p+1 end);
                # lazy = v block 1 (deadline: pair 4) ----
                urgent = deque()
                lazy = deque()
                for p in range(HPAIRS):
                    if p + 2 < HPAIRS:
                        urgent.extend(qk_units(b, p + 2))
                    if p == 0:
                        lazy.extend(v_units(b, 1))
                    if p >= 2 and p % 2 == 0:
                        e_pre = prefix_group(p // 2)
                    lazy_budget = 2
                    slot = 0
                    for hh in range(2):
                        base = hh * 64
                        h = 2 * p + hh
                        ps_av = psAV.tile([128, N], F32, tag="av",
                                          name=f"av_{b}_{h}")
                        # prefix contribution from the shared packed exp
                        for j in (0, 512):
                            nc.tensor.matmul(
                                ps_av[:, j:j + 512],
                                v_ext[:, 0, h, :],
                                e_pre[:, j:j + 512],
                                start=True, stop=False,
                            )
                        for mt in range(1, MT):
                            ps_s = psS.tile([128, N], F32, tag="s",
                                            name=f"s_{b}_{h}_{mt}")
                            for j in (0, 512):
                                nc.tensor.matmul(
                                    ps_s[:, j:j + 512],
                                    kT[base:base + D, p % 4,
                                       (mt - 1) * 128:mt * 128],
                                    qT[base:base + D, p % 4, j:j + 512],
                                    start=True, stop=True,
                                )
                            eT = e_pool.tile([128, N], BF16, tag="e",
                                             name=f"e_{b}_{h}_{mt}")
                            nc.scalar.activation(eT[:], ps_s[:], AF.Exp,
                                                 scale=SCALE)
                            # gemm/proj filler BETWEEN exp and av: the PE
                            # would otherwise idle waiting for the exp (and,
                            # at mt==1, for the previous head's psum release)
                            slot += 1
                            if urgent and (mt in (1, 5)
                                           or len(urgent) >= 18 - slot):
                                urgent.popleft()()
                            elif lazy and lazy_budget > 0 and mt in (3, 7):
                                lazy.popleft()()
                                lazy_budget -= 1
                            for j in (0, 512):
                                nc.tensor.matmul(
                                    ps_av[:, j:j + 512],
                                    v_ext[:, mt, h, :],
                                    eT[:, j:j + 512],
                                    start=False, stop=(mt == MT - 1),
                                )
                        # normalize: out = unnorm * exp(-ln(denom)).
                        # (custom-DVE reciprocal_approx is unsupported by this
                        # walrus; iterative DVE reciprocal costs 6.5us.)
                        # The numerator is copied to SBUF so the psum
                        # accumulator is released after ~1.1us (copy || ln)
                        # instead of after the full ln->exp->mul chain.
                        num_sb = stg.tile([64, N], F32, tag="st",
                                          name=f"st_{b}_{h}")
                        nc.vector.tensor_copy(num_sb[:], ps_av[0:64, :])
                        lnd = rb_pool.tile([64, N], F32, tag="ln",
                                           name=f"ln_{b}_{h}")
                        nc.scalar.activation(lnd[:], ps_av[64:128, :], AF.Ln)
                        rb = rb_pool.tile([64, N], F32, tag="rb",
                                          name=f"rb_{b}_{h}")
                        nc.scalar.activation(rb[:], lnd[:], AF.Exp,
                                             scale=-1.0)
                        nc.vector.tensor_mul(
                            oT[base:base + D, p, :], num_sb[:], rb[:]
                        )
                    if p >= HPAIRS - 3:
                        # tail: no further slots are guaranteed, flush
                        while urgent:
                            urgent.popleft()()
                        while lazy:
                            lazy.popleft()()

                return proj_units(b)

            carry = []
            for _rep in range(repeat):
                for b in range(B_PC):
                    carry = emit_batch(b, carry)
            for u in carry:
                u()

    return nc


_NC_CACHE = {}


def _get_nc(repeat: int = 1) -> bass.Bass:
    key = f"nc{repeat}"
    if key not in _NC_CACHE:
        _NC_CACHE[key] = build_nc(repeat)
    return _NC_CACHE[key]


def _make_runner(nc):
    """Compile the SPMD kernel ONCE into a reusable callable.

    Mirrors bass2jax.run_bass_via_pjrt's multi-core branch, but without
    output-buffer donation so the compiled function + device-resident
    inputs can be invoked repeatedly (for wall-clock benchmarking and to
    avoid recompiles on every kernel() call).
    """
    import jax
    from jax.experimental.shard_map import shard_map
    from jax.sharding import Mesh, PartitionSpec
    from concourse import bass2jax
    from concourse.bass2jax import _bass_exec_p, partition_id_tensor

    bass2jax.install_neuronx_cc_hook()

    partition_name = (
        nc.partition_id_tensor.name if nc.partition_id_tensor else None
    )
    in_names, out_names, out_avals, zero_outs = [], [], [], []
    for alloc in nc.m.functions[0].allocations:
        if not isinstance(alloc, mybir.MemoryLocationSet):
            continue
        name = alloc.memorylocations[0].name
        if alloc.kind == "ExternalInput":
            if name != partition_name:
                in_names.append(name)
        elif alloc.kind == "ExternalOutput":
            shape = tuple(alloc.tensor_shape)
            dtype = mybir.dt.np(alloc.dtype)
            out_names.append(name)
            out_avals.append(jax.core.ShapedArray(shape, dtype))
            zero_outs.append(np.zeros(shape, dtype))
    n_params = len(in_names)
    all_in_names = list(in_names) + list(out_names)
    if partition_name is not None:
        all_in_names.append(partition_name)

    def _body(*args):
        operands = list(args)
        if partition_name is not None:
            operands.append(partition_id_tensor())
        outs = _bass_exec_p.bind(
            *operands,
            out_avals=tuple(out_avals),
            in_names=tuple(all_in_names),
            out_names=tuple(out_names),
            lowering_input_output_aliases=(),
            sim_require_finite=True,
            sim_require_nnan=True,
            nc=nc,
        )
        return tuple(outs)

    devices = jax.devices()[:N_CORES]
    mesh = Mesh(np.asarray(devices), ("core",))
    n_outs = len(out_avals)
    in_specs = (PartitionSpec("core"),) * (n_params + n_outs)
    out_specs = (PartitionSpec("core"),) * n_outs
    sharded = jax.jit(
        shard_map(_body, mesh=mesh, in_specs=in_specs,
                  out_specs=out_specs, check_rep=False),
        keep_unused=True,
    )

    concat_zeros = [
        np.zeros((N_CORES * z.shape[0], *z.shape[1:]), z.dtype)
        for z in zero_outs
    ]

    state = {"dev_zeros": None}

    def runner(in_maps):
        per_core = [
            [np.asarray(m[name]) for name in in_names] for m in in_maps
        ]
        concat_in = [
            np.concatenate([per_core[c][i] for c in range(N_CORES)], axis=0)
            for i in range(n_params)
        ]
        if state["dev_zeros"] is None:
            state["dev_zeros"] = [jax.device_put(z) for z in concat_zeros]
        out_arrs = sharded(*concat_in, *state["dev_zeros"])
        return [
            {
                name: np.asarray(out_arrs[i]).reshape(
                    N_CORES, *out_avals[i].shape
                )[c]
                for i, name in enumerate(out_names)
            }
            for c in range(N_CORES)
        ]

    def runner_dev(dev_args):
        """dev_args: device-resident concat inputs; returns device outputs."""
        return sharded(*dev_args, *state["dev_zeros"])

    def make_dev_args(in_maps):
        per_core = [
            [np.asarray(m[name]) for name in in_names] for m in in_maps
        ]
        concat_in = [
            np.concatenate([per_core[c][i] for c in range(N_CORES)], axis=0)
            for i in range(n_params)
        ]
        if state["dev_zeros"] is None:
            state["dev_zeros"] = [jax.device_put(z) for z in concat_zeros]
        return [jax.device_put(a) for a in concat_in]

    return runner, runner_dev, make_dev_args


def _get_runner(repeat: int = 1):
    key = f"runner{repeat}"
    if key not in _NC_CACHE:
        _NC_CACHE[key] = _make_runner(_get_nc(repeat))
    return _NC_CACHE[key]


def _make_in_maps(x, pk, pv, w_qkv, w_proj, b_proj):
    x = np.ascontiguousarray(np.asarray(x, dtype=np.float32))
    pk = np.ascontiguousarray(np.asarray(pk, dtype=np.float32))
    pv = np.ascontiguousarray(np.asarray(pv, dtype=np.float32))
    w_qkv = np.ascontiguousarray(np.asarray(w_qkv, dtype=np.float32))
    w_proj = np.ascontiguousarray(np.asarray(w_proj, dtype=np.float32))
    b_proj = np.ascontiguousarray(np.asarray(b_proj, dtype=np.float32))
    in_maps = []
    for c in range(N_CORES):
        sl = slice(c * B_PC, (c + 1) * B_PC)
        in_maps.append({
            "x": x[sl], "pk": pk[sl], "pv": pv[sl],
            "w_qkv": w_qkv, "w_proj": w_proj, "b_proj": b_proj,
        })
    return in_maps


def run(x, pk, pv, w_qkv, w_proj, b_proj, trace=False, **trace_kwargs):
    """Run the SPMD kernel; returns (output [B,N,C], results).

    With trace=True, routes through run_bass_kernel_spmd so the returned
    results object carries .exec_time_ns / .profile_json.
    """
    in_maps = _make_in_maps(x, pk, pv, w_qkv, w_proj, b_proj)
    if trace:
        res = run_bass_kernel_spmd(
            _get_nc(), in_maps, list(range(N_CORES)), trace=True,
            **trace_kwargs,
        )
        results = res.results
        out = np.empty((B, N, C), dtype=np.float32)
        for c in range(N_CORES):
            outT = results[c]["outT"]          # [B_PC, C, N]
            out[c * B_PC:(c + 1) * B_PC] = outT.transpose(0, 2, 1)
        return out, res
    runner, _, _ = _get_runner()
    results = runner(in_maps)
    out = np.empty((B, N, C), dtype=np.float32)
    for c in range(N_CORES):
        outT = results[c]["outT"]              # [B_PC, C, N]
        out[c * B_PC:(c + 1) * B_PC] = outT.transpose(0, 2, 1)
    return out, results


def kernel(x, pk, pv, w_qkv, w_proj, b_proj) -> np.ndarray:
    out, _ = run(x, pk, pv, w_qkv, w_proj, b_proj)
    return out


def benchmark(x, pk, pv, w_qkv, w_proj, b_proj, iters=20, warmup=3, repeat=1):
    """Median wall-clock per executed call with device-resident inputs."""
    import time
    import jax
    _, runner_dev, make_dev_args = _get_runner(repeat)
    in_maps = _make_in_maps(x, pk, pv, w_qkv, w_proj, b_proj)
    dev_args = make_dev_args(in_maps)
    for _ in range(warmup):
        outs = runner_dev(dev_args)
        jax.block_until_ready(outs)
    ts = []
    for _ in range(iters):
        t0 = time.perf_counter()
        outs = runner_dev(dev_args)
        jax.block_until_ready(outs)
        ts.append(time.perf_counter() - t0)
    ts.sort()
    return {
        "median_s": ts[len(ts) // 2],
        "min_s": ts[0],
        "all_s": ts,
    }

